# revision 18
# baseline (speedup 1.0000x reference)
"""Trainium2 Bass kernel: ConvLSTM1D -> BiLSTM -> dense sigmoid.

Reference model (per full batch B=32):
  h = ConvLSTM1D(x (B,64,512,32); k (2,32,128) stride2, r (2,32,128), hard_sigmoid)
      -> final hidden (B, 256, 32)
  hf = LSTM(h) last state; hb = LSTM(h reversed) last state  (U=32 each)
  out = sigmoid(concat(hf,hb) @ w_d + b_d)   (B, 1)

Sharding: pure data parallelism, batch 32 -> 8 cores x 4.

Per-core layout choices:
  ConvLSTM scan state/gates: partitions = (b4, ch32) = 128, free = j (256).
    Matmuls use block-diagonal weights lhsT[(b',cin),(b,ch)] = delta_bb' W[cin,ch]
    (K=128, M=128, N=256, float32r -> 1 cycle/row) accumulating input-conv taps
    and recurrent-conv taps into one PSUM group per gate.
  BiLSTM: transposed layout, partitions = (gate,U) = 128, free = batch (4).
    Two interleaved chains (fwd, bwd); zx injected by identity-matmul.
Gate order is host-reordered from Keras (i,f,g,o) to (i,f,o,g) so the three
hard-sigmoid/sigmoid gates are contiguous.
"""

import numpy as np

import concourse.bass as bass
import concourse.bacc as bacc
import concourse.mybir as mybir
from concourse.tile import TileContext
from concourse.bass_utils import run_bass_kernel_spmd

B, T, L, C = 32, 64, 512, 32
F = 32          # conv filters
U = 32          # lstm units
NCORES = 8
BL = B // NCORES          # 4 local batch
LO = L // 2               # 256 spatial after stride-2 conv
G4 = 4 * F                # 128 gate channels

FP = mybir.dt.float32
BF = mybir.dt.bfloat16

# w_bf column layout (bf16): big matmul weights
#  [0:2048)    16 block-diag (128x128) conv weights, index (g*2+tap)*128,
#              first 8 = input conv, next 8 = recurrent conv
#  [2048:2176) identity 128x128
#  [2176:3200) 8 block-diag zx weights bdk[d][g][(b,ch),(b,U)]
#  [3200:4224) 8 block-diag lstm rec weights bdr[d][g][(b,U'),(b,U)]
#  [4224:4232) dense wdx[d] (128,4): [(b,u), b] = delta * w_d[u+32d]
WBF_COLS = 4232
# w_all column layout (f32): biases
#  [0:8)       lstm biases per (d,g): (128,1) = b_d[g*32+u]
#  [8]         0.5 constant
#  [9]         b_d (dense bias) replicated
W_COLS = 10

_CACHE = {}


def _reorder_gates(w, n):
    # last dim (4n): keras order i,f,g,o -> i,f,o,g
    i, f, g, o = np.split(w, 4, axis=-1)
    return np.concatenate([i, f, o, g], axis=-1)


def _build_graph():
    nc = bacc.Bacc("TRN2")
    x2 = nc.declare_dram_parameter("x2", [128, T, 2 * LO], BF, isOutput=False)
    w_bf = nc.declare_dram_parameter("w_bf", [128, WBF_COLS], BF, isOutput=False)
    w_all = nc.declare_dram_parameter("w_all", [128, W_COLS], FP, isOutput=False)
    out = nc.declare_dram_parameter("out", [BL, 1], FP, isOutput=True)

    AF = mybir.ActivationFunctionType
    ALU = mybir.AluOpType

    with TileContext(nc) as tc:
        with (
            tc.tile_pool(name="w", bufs=1) as wp,
            tc.tile_pool(name="x", bufs=4) as xp,
            tc.tile_pool(name="st", bufs=1) as sp,
            tc.tile_pool(name="g", bufs=3) as gp,
            tc.tile_pool(name="zp", bufs=2, space="PSUM") as zp,
            tc.tile_pool(name="sc", bufs=1, space="PSUM") as scp,
        ):
            W = wp.tile([128, W_COLS], FP)
            nc.sync.dma_start(out=W[:], in_=w_all[:])
            WB = wp.tile([128, WBF_COLS], BF)
            nc.sync.dma_start(out=WB[:], in_=w_bf[:])

            def wconv(idx):  # (128,128) bf16 block-diag conv weight
                return WB[:, idx * 128:(idx + 1) * 128]

            ident = WB[:, 2048:2176]

            def bdk(d, g):  # zx input weights, block-diag (bf16)
                o = 2176 + (d * 4 + g) * 128
                return WB[:, o:o + 128]

            def bdr(d, g):  # lstm recurrent weights, block-diag (bf16)
                o = 3200 + (d * 4 + g) * 128
                return WB[:, o:o + 128]

            wdx = [WB[:, 4224:4228], WB[:, 4228:4232]]
            bls = [[W[:, d * 4 + g:d * 4 + g + 1] for g in range(4)]
                   for d in range(2)]
            half = W[:, 8:9]
            bd = W[0:4, 9:10]

            # ---------------- Phase A: ConvLSTM scan over T ----------------
            h_sb = sp.tile([128, LO + 1], BF)   # col 256 stays zero (pad)
            c_sb = sp.tile([128, LO], FP)
            nc.vector.memset(h_sb[:, LO:LO + 1], 0.0)

            # gate column order in z: g, i, f, o (g first so tanh_g and
            # tmp can start while later gates' matmuls still run)
            GPOS = {"g": 0, "f": 1, "i": 2, "o": 3}
            for t in range(T):
                xt = xp.tile([128, 2, LO], BF, tag="xt")
                nc.sync.dma_start(out=xt[:], in_=x2[:, t, :])
                z = zp.tile([128, 4 * LO], FP, tag="z")
                sig = gp.tile([128, 3, LO], BF, tag="sig")
                tg = gp.tile([128, LO], BF, tag="tg")
                tc_t = gp.tile([128, LO], BF, tag="tc")
                tmp = gp.tile([128, LO], BF, tag="tmp")
                c2 = gp.tile([128, LO], FP, tag="c2")

                def convmm(g, pos):
                    zg = z[:, pos * LO:(pos + 1) * LO]
                    n_acc = 4 if t > 0 else 2
                    k = 0
                    for tap in range(2):
                        nc.tensor.matmul(
                            zg, lhsT=wconv(g * 2 + tap), rhs=xt[:, tap, :],
                            start=(k == 0), stop=(k == n_acc - 1))
                        k += 1
                    if t > 0:
                        for tap in range(2):
                            nc.tensor.matmul(
                                zg, lhsT=wconv(8 + g * 2 + tap),
                                rhs=h_sb[:, tap:tap + LO],
                                start=False, stop=(k == n_acc - 1))
                            k += 1

                # gate index in weights: 0=i 1=f 2=o 3=g (host order i,f,o,g)
                convmm(3, GPOS["g"])
                nc.scalar.activation(tg[:], z[:, 0:LO], AF.Tanh)
                convmm(0, GPOS["i"])
                nc.scalar.activation(sig[:, 0, :], z[:, 2 * LO:3 * LO],
                                     AF.Relu, bias=half, scale=0.2)
                # tmp = min(sig_i,1) * tanh(zg)
                nc.vector.scalar_tensor_tensor(
                    (c_sb[:] if t == 0 else tmp[:]),
                    sig[:, 0, :], 1.0, tg[:], ALU.min, ALU.mult)
                convmm(1, GPOS["f"])
                nc.scalar.activation(sig[:, 1, :], z[:, LO:2 * LO],
                                     AF.Relu, bias=half, scale=0.2)
                if t > 0:
                    nc.vector.scalar_tensor_tensor(
                        c2[:], sig[:, 1, :], 1.0, c_sb[:], ALU.min, ALU.mult)
                    nc.vector.tensor_tensor(c_sb[:], tmp[:], c2[:], ALU.add)
                convmm(2, GPOS["o"])
                nc.scalar.activation(sig[:, 2, :], z[:, 3 * LO:4 * LO],
                                     AF.Relu, bias=half, scale=0.2)
                nc.scalar.activation(tc_t[:], c_sb[:], AF.Tanh)
                nc.vector.scalar_tensor_tensor(
                    h_sb[:, 0:LO], sig[:, 2, :], 1.0, tc_t[:],
                    ALU.min, ALU.mult)

            # ---------------- Phase B: bidirectional LSTM over LO ----------
            # Layout: partitions = (b,U) = 128, free = gate cols. No partition
            # shifts anywhere (walrus verifier requires same partitions).
            # zx[d][g] (128, LO): input-side gates + lstm bias, injected into
            # the per-step PSUM via identity matmul (i,f,o) / ACT bias (g).
            zxs = []
            for d in range(2):
                ps = zp.tile([128, 4 * LO], FP, tag="z", name=f"zxps{d}")
                for g in range(4):
                    nc.tensor.matmul(
                        ps[:, g * LO:(g + 1) * LO], lhsT=bdk(d, g),
                        rhs=h_sb[:, 0:LO],
                        start=True, stop=True)
                zx_ifo = sp.tile([128, LO, 3], BF, tag=f"zxifo{d}",
                                 name=f"zxifo{d}")
                zx_g = sp.tile([128, LO], FP, tag=f"zxg{d}", name=f"zxg{d}")
                # evacuation + lstm-bias fold; split across ACT and DVE
                nc.scalar.activation(
                    zx_ifo[:, :, 0], ps[:, 0:LO], AF.Identity, bias=bls[d][0])
                nc.vector.scalar_tensor_tensor(
                    zx_ifo[:, :, 1], ps[:, LO:2 * LO], bls[d][1],
                    h_sb[:, 0:LO], ALU.add, ALU.bypass)
                nc.scalar.activation(
                    zx_ifo[:, :, 2], ps[:, 2 * LO:3 * LO], AF.Identity,
                    bias=bls[d][2])
                nc.vector.scalar_tensor_tensor(
                    zx_g[:], ps[:, 3 * LO:4 * LO], bls[d][3],
                    h_sb[:, 0:LO], ALU.add, ALU.bypass)
                zxs.append((zx_ifo, zx_g))

            # state: hT[d] bf16 (feeds bf16 matmul), cT[d] f32
            hT = [sp.tile([128, 1], BF, tag=f"hT{d}", name=f"hT{d}")
                  for d in range(2)]
            cT = [sp.tile([128, 1], FP, tag=f"cT{d}", name=f"cT{d}")
                  for d in range(2)]
            # 4 PSUM banks: region per (dir, parity) so PE writes of one
            # step never share a bank with ACT reads of the previous step
            zt = scp.tile([128, 2048], FP, tag="zt")

            for s in range(LO):
                for d in range(2):
                    se = s if d == 0 else LO - 1 - s
                    zx_ifo, zx_g = zxs[d]
                    c0 = (d * 2 + (s % 2)) * 512
                    zifo = zt[:, c0:c0 + 3]
                    zg = zt[:, c0 + 3:c0 + 4]
                    nc.tensor.matmul(zifo, lhsT=ident,
                                     rhs=zx_ifo[:, se, :],
                                     start=True, stop=(s == 0),
                                     skip_group_check=True)
                    if s > 0:
                        for g in range(3):
                            nc.tensor.matmul(
                                zt[:, c0 + g:c0 + g + 1], lhsT=bdr(d, g),
                                rhs=hT[d][:], start=False, stop=(g == 2),
                                skip_group_check=True)
                        nc.tensor.matmul(zg, lhsT=bdr(d, 3), rhs=hT[d][:],
                                         start=True, stop=True)
                        tg_b = zx_g[:, se:se + 1]
                    else:
                        tg_b = None
                    sg = gp.tile([128, 3], BF, tag=f"sg{d}")
                    tgl = gp.tile([128, 1], BF, tag=f"tg{d}")
                    tcl = gp.tile([128, 1], BF, tag=f"tc{d}")
                    tm1 = gp.tile([128, 1], FP, tag=f"tm1{d}")
                    nc.scalar.activation(sg[:], zifo, AF.Sigmoid)
                    if s > 0:
                        nc.scalar.activation(tgl[:], zg, AF.Tanh, bias=tg_b)
                    else:
                        nc.scalar.activation(tgl[:], zx_g[:, se:se + 1],
                                             AF.Tanh)
                    # tm1 = sig_i * tanh_g   (tgl as per-partition scalar)
                    nc.gpsimd.tensor_tensor(tm1[:], sg[:, 0:1], tgl[:],
                                            ALU.mult)
                    if s > 0:
                        # c = sig_f * c + tm1  (cT as per-partition scalar)
                        nc.vector.scalar_tensor_tensor(
                            cT[d][:], sg[:, 1:2], cT[d][:], tm1[:],
                            ALU.mult, ALU.add)
                    else:
                        nc.vector.tensor_copy(cT[d][:], tm1[:])
                    nc.scalar.activation(tcl[:], cT[d][:], AF.Tanh)
                    # h = sig_o * tanh_c
                    nc.vector.scalar_tensor_tensor(
                        hT[d][:], sg[:, 2:3], tcl[:], sg[:, 2:3],
                        ALU.mult, ALU.bypass)

            # ---------------- dense + sigmoid ----------------
            fo = zt[0:BL, 2044:2045]
            nc.tensor.matmul(fo, lhsT=wdx[0], rhs=hT[0][:],
                             start=True, stop=False, skip_group_check=True)
            nc.tensor.matmul(fo, lhsT=wdx[1], rhs=hT[1][:],
                             start=False, stop=True, skip_group_check=True)
            res = gp.tile([BL, 1], FP, tag="res")
            nc.scalar.activation(res[:], fo, AF.Sigmoid, bias=bd)
            nc.sync.dma_start(out=out[:], in_=res[:])

    nc.compile()
    return nc


def _prep_inputs(x, k_conv, r_conv, b_conv, k_f, r_f, b_f, k_b, r_b, b_b,
                 w_d, b_d):
    """Host-side: gate reorder, block-diag expansion, x transpose."""
    assert np.all(b_conv == 0.0), "nonzero b_conv not supported by this kernel"
    k_conv = _reorder_gates(np.asarray(k_conv, np.float32), F)
    r_conv = _reorder_gates(np.asarray(r_conv, np.float32), F)
    k_f = _reorder_gates(np.asarray(k_f, np.float32), U)
    r_f = _reorder_gates(np.asarray(r_f, np.float32), U)
    b_f = _reorder_gates(np.asarray(b_f, np.float32), U)
    k_b = _reorder_gates(np.asarray(k_b, np.float32), U)
    r_b = _reorder_gates(np.asarray(r_b, np.float32), U)
    b_b = _reorder_gates(np.asarray(b_b, np.float32), U)

    import ml_dtypes
    w_bf = np.zeros((128, WBF_COLS), np.float32)
    w_all = np.zeros((128, W_COLS), np.float32)
    for g in range(4):
        for tap in range(2):
            wi = np.zeros((128, 128), np.float32)
            wr = np.zeros((128, 128), np.float32)
            for b in range(4):
                sl = slice(b * 32, (b + 1) * 32)
                wi[sl, sl] = k_conv[tap, :, g * 32:(g + 1) * 32]
                wr[sl, sl] = r_conv[tap, :, g * 32:(g + 1) * 32]
            w_bf[:, (g * 2 + tap) * 128:(g * 2 + tap + 1) * 128] = wi
            w_bf[:, (8 + g * 2 + tap) * 128:(9 + g * 2 + tap) * 128] = wr
    w_bf[:, 2048:2176] = np.eye(128, dtype=np.float32)
    w_d = np.asarray(w_d, np.float32)
    for d, (kk, rr, bb) in enumerate([(k_f, r_f, b_f), (k_b, r_b, b_b)]):
        for g in range(4):
            bk = np.zeros((128, 128), np.float32)
            br = np.zeros((128, 128), np.float32)
            for b in range(4):
                sl = slice(b * 32, (b + 1) * 32)
                bk[sl, sl] = kk[:, g * 32:(g + 1) * 32]
                br[sl, sl] = rr[:, g * 32:(g + 1) * 32]
            w_bf[:, 2176 + (d * 4 + g) * 128:2304 + (d * 4 + g) * 128] = bk
            w_bf[:, 3200 + (d * 4 + g) * 128:3328 + (d * 4 + g) * 128] = br
            w_all[:, d * 4 + g] = np.tile(bb[g * 32:(g + 1) * 32], 4)
        wx = np.zeros((128, 4), np.float32)
        for b in range(4):
            wx[b * 32:(b + 1) * 32, b] = w_d[d * 32:(d + 1) * 32, 0]
        w_bf[:, 4224 + d * 4:4228 + d * 4] = wx
    w_all[:, 8] = 0.5
    w_all[0:4, 9] = np.float32(np.asarray(b_d).reshape(-1)[0])
    w_bf = w_bf.astype(ml_dtypes.bfloat16)

    # x (B,T,512,C) -> per-core (128=(b,c), T, (tap,j)): x2[b*32+c, t, tap*256+j]
    #   = x[b, t, 2j+tap, c]
    x = np.asarray(x, np.float32).reshape(B, T, LO, 2, C)
    # -> (B, C, T, tap, j)
    xt = np.ascontiguousarray(x.transpose(0, 4, 1, 3, 2))
    x2_full = xt.reshape(B * C, T, 2 * LO)

    x2_full = x2_full.astype(ml_dtypes.bfloat16)
    in_maps = []
    for core in range(NCORES):
        x2c = np.ascontiguousarray(
            x2_full[core * BL * C:(core + 1) * BL * C])
        in_maps.append({"x2": x2c, "w_bf": w_bf, "w_all": w_all})
    return in_maps


def kernel(**inputs) -> np.ndarray:
    if "nc" not in _CACHE:
        _CACHE["nc"] = _build_graph()
    nc = _CACHE["nc"]
    in_maps = _prep_inputs(**inputs)
    res = run_bass_kernel_spmd(nc, in_maps, core_ids=list(range(NCORES)))
    outs = [res.results[i]["out"].reshape(BL, 1) for i in range(NCORES)]
    return np.concatenate(outs, axis=0).astype(np.float32)


# revision 19
# speedup vs baseline: 1.0026x; 1.0026x over previous
"""Trainium2 Bass kernel: ConvLSTM1D -> BiLSTM -> dense sigmoid.

Reference model (per full batch B=32):
  h = ConvLSTM1D(x (B,64,512,32); k (2,32,128) stride2, r (2,32,128), hard_sigmoid)
      -> final hidden (B, 256, 32)
  hf = LSTM(h) last state; hb = LSTM(h reversed) last state  (U=32 each)
  out = sigmoid(concat(hf,hb) @ w_d + b_d)   (B, 1)

Sharding: pure data parallelism, batch 32 -> 8 cores x 4.

Per-core layout choices:
  ConvLSTM scan state/gates: partitions = (b4, ch32) = 128, free = j (256).
    Matmuls use block-diagonal weights lhsT[(b',cin),(b,ch)] = delta_bb' W[cin,ch]
    (K=128, M=128, N=256, float32r -> 1 cycle/row) accumulating input-conv taps
    and recurrent-conv taps into one PSUM group per gate.
  BiLSTM: transposed layout, partitions = (gate,U) = 128, free = batch (4).
    Two interleaved chains (fwd, bwd); zx injected by identity-matmul.
Gate order is host-reordered from Keras (i,f,g,o) to (i,f,o,g) so the three
hard-sigmoid/sigmoid gates are contiguous.
"""

import numpy as np

import concourse.bass as bass
import concourse.bacc as bacc
import concourse.mybir as mybir
from concourse.tile import TileContext
from concourse.bass_utils import run_bass_kernel_spmd

B, T, L, C = 32, 64, 512, 32
F = 32          # conv filters
U = 32          # lstm units
NCORES = 8
BL = B // NCORES          # 4 local batch
LO = L // 2               # 256 spatial after stride-2 conv
G4 = 4 * F                # 128 gate channels

FP = mybir.dt.float32
BF = mybir.dt.bfloat16

# w_bf column layout (bf16): big matmul weights
#  [0:2048)    16 block-diag (128x128) conv weights, index (g*2+tap)*128,
#              first 8 = input conv, next 8 = recurrent conv
#  [2048:2176) identity 128x128
#  [2176:3200) 8 block-diag zx weights bdk[d][g][(b,ch),(b,U)]
#  [3200:4224) 8 block-diag lstm rec weights bdr[d][g][(b,U'),(b,U)]
#  [4224:4232) dense wdx[d] (128,4): [(b,u), b] = delta * w_d[u+32d]
WBF_COLS = 4232
# w_all column layout (f32): biases
#  [0:8)       lstm biases per (d,g): (128,1) = b_d[g*32+u]
#  [8]         0.5 constant
#  [9]         b_d (dense bias) replicated
W_COLS = 10

_CACHE = {}


def _reorder_gates(w, n):
    # last dim (4n): keras order i,f,g,o -> i,f,o,g
    i, f, g, o = np.split(w, 4, axis=-1)
    return np.concatenate([i, f, o, g], axis=-1)


def _build_graph():
    nc = bacc.Bacc("TRN2")
    x2 = nc.declare_dram_parameter("x2", [128, T, 2 * LO], BF, isOutput=False)
    w_bf = nc.declare_dram_parameter("w_bf", [128, WBF_COLS], BF, isOutput=False)
    w_all = nc.declare_dram_parameter("w_all", [128, W_COLS], FP, isOutput=False)
    out = nc.declare_dram_parameter("out", [BL, 1], FP, isOutput=True)

    AF = mybir.ActivationFunctionType
    ALU = mybir.AluOpType

    with TileContext(nc) as tc:
        with (
            tc.tile_pool(name="w", bufs=1) as wp,
            tc.tile_pool(name="x", bufs=4) as xp,
            tc.tile_pool(name="st", bufs=1) as sp,
            tc.tile_pool(name="g", bufs=3) as gp,
            tc.tile_pool(name="zp", bufs=2, space="PSUM") as zp,
            tc.tile_pool(name="sc", bufs=1, space="PSUM") as scp,
        ):
            W = wp.tile([128, W_COLS], FP)
            nc.sync.dma_start(out=W[:], in_=w_all[:])
            WB = wp.tile([128, WBF_COLS], BF)
            nc.sync.dma_start(out=WB[:], in_=w_bf[:])

            def wconv(idx):  # (128,128) bf16 block-diag conv weight
                return WB[:, idx * 128:(idx + 1) * 128]

            ident = WB[:, 2048:2176]

            def bdk(d, g):  # zx input weights, block-diag (bf16)
                o = 2176 + (d * 4 + g) * 128
                return WB[:, o:o + 128]

            def bdr(d, g):  # lstm recurrent weights, block-diag (bf16)
                o = 3200 + (d * 4 + g) * 128
                return WB[:, o:o + 128]

            wdx = [WB[:, 4224:4228], WB[:, 4228:4232]]
            bls = [[W[:, d * 4 + g:d * 4 + g + 1] for g in range(4)]
                   for d in range(2)]
            half = W[:, 8:9]
            bd = W[0:4, 9:10]

            # ---------------- Phase A: ConvLSTM scan over T ----------------
            h_sb = sp.tile([128, LO + 1], BF)   # col 256 stays zero (pad)
            c_sb = sp.tile([128, LO], FP)
            nc.vector.memset(h_sb[:, LO:LO + 1], 0.0)

            # gate column order in z: g, i, f, o (g first so tanh_g and
            # tmp can start while later gates' matmuls still run)
            GPOS = {"g": 0, "f": 1, "i": 2, "o": 3}
            for t in range(T):
                xt = xp.tile([128, 2, LO], BF, tag="xt")
                nc.sync.dma_start(out=xt[:], in_=x2[:, t, :])
                z = zp.tile([128, 4 * LO], FP, tag="z")
                sig = gp.tile([128, 3, LO], BF, tag="sig")
                tg = gp.tile([128, LO], BF, tag="tg")
                tc_t = gp.tile([128, LO], BF, tag="tc")
                tmp = gp.tile([128, LO], BF, tag="tmp")
                c2 = gp.tile([128, LO], FP, tag="c2")

                def convmm(g, pos):
                    zg = z[:, pos * LO:(pos + 1) * LO]
                    n_acc = 4 if t > 0 else 2
                    k = 0
                    for tap in range(2):
                        nc.tensor.matmul(
                            zg, lhsT=wconv(g * 2 + tap), rhs=xt[:, tap, :],
                            start=(k == 0), stop=(k == n_acc - 1))
                        k += 1
                    if t > 0:
                        for tap in range(2):
                            nc.tensor.matmul(
                                zg, lhsT=wconv(8 + g * 2 + tap),
                                rhs=h_sb[:, tap:tap + LO],
                                start=False, stop=(k == n_acc - 1))
                            k += 1

                # gate index in weights: 0=i 1=f 2=o 3=g (host order i,f,o,g)
                convmm(3, GPOS["g"])
                nc.scalar.activation(tg[:], z[:, 0:LO], AF.Tanh)
                convmm(0, GPOS["i"])
                nc.scalar.activation(sig[:, 0, :], z[:, 2 * LO:3 * LO],
                                     AF.Relu, bias=half, scale=0.2)
                # tmp = min(sig_i,1) * tanh(zg)
                nc.vector.scalar_tensor_tensor(
                    (c_sb[:] if t == 0 else tmp[:]),
                    sig[:, 0, :], 1.0, tg[:], ALU.min, ALU.mult)
                convmm(1, GPOS["f"])
                nc.scalar.activation(sig[:, 1, :], z[:, LO:2 * LO],
                                     AF.Relu, bias=half, scale=0.2)
                if t > 0:
                    nc.vector.scalar_tensor_tensor(
                        c2[:], sig[:, 1, :], 1.0, c_sb[:], ALU.min, ALU.mult)
                    nc.vector.tensor_tensor(c_sb[:], tmp[:], c2[:], ALU.add)
                convmm(2, GPOS["o"])
                nc.scalar.activation(sig[:, 2, :], z[:, 3 * LO:4 * LO],
                                     AF.Relu, bias=half, scale=0.2)
                nc.scalar.activation(tc_t[:], c_sb[:], AF.Tanh)
                nc.vector.scalar_tensor_tensor(
                    h_sb[:, 0:LO], sig[:, 2, :], 1.0, tc_t[:],
                    ALU.min, ALU.mult)

            # ---------------- Phase B: bidirectional LSTM over LO ----------
            # Layout: partitions = (b,U) = 128, free = gate cols. No partition
            # shifts anywhere (walrus verifier requires same partitions).
            # zx[d][g] (128, LO): input-side gates + lstm bias, injected into
            # the per-step PSUM via identity matmul (i,f,o) / ACT bias (g).
            zxs = []
            for d in range(2):
                ps = zp.tile([128, 4 * LO], FP, tag="z", name=f"zxps{d}")
                for g in range(4):
                    nc.tensor.matmul(
                        ps[:, g * LO:(g + 1) * LO], lhsT=bdk(d, g),
                        rhs=h_sb[:, 0:LO],
                        start=True, stop=True)
                zx_ifo = sp.tile([128, LO, 3], BF, tag=f"zxifo{d}",
                                 name=f"zxifo{d}")
                zx_g = sp.tile([128, LO], FP, tag=f"zxg{d}", name=f"zxg{d}")
                # evacuation + lstm-bias fold; split across ACT and DVE
                nc.scalar.activation(
                    zx_ifo[:, :, 0], ps[:, 0:LO], AF.Identity, bias=bls[d][0])
                nc.vector.scalar_tensor_tensor(
                    zx_ifo[:, :, 1], ps[:, LO:2 * LO], bls[d][1],
                    h_sb[:, 0:LO], ALU.add, ALU.bypass)
                nc.scalar.activation(
                    zx_ifo[:, :, 2], ps[:, 2 * LO:3 * LO], AF.Identity,
                    bias=bls[d][2])
                nc.vector.scalar_tensor_tensor(
                    zx_g[:], ps[:, 3 * LO:4 * LO], bls[d][3],
                    h_sb[:, 0:LO], ALU.add, ALU.bypass)
                zxs.append((zx_ifo, zx_g))

            # state: hT[d] bf16 (feeds bf16 matmul), cT[d] f32
            hT = [sp.tile([128, 1], BF, tag=f"hT{d}", name=f"hT{d}")
                  for d in range(2)]
            cT = [sp.tile([128, 1], FP, tag=f"cT{d}", name=f"cT{d}")
                  for d in range(2)]
            # 4 PSUM banks: region per (dir, parity) so PE writes of one
            # step never share a bank with ACT reads of the previous step
            zt = scp.tile([128, 2048], FP, tag="zt")

            for s in range(LO):
                for d in range(2):
                    se = s if d == 0 else LO - 1 - s
                    zx_ifo, zx_g = zxs[d]
                    c0 = (d * 2 + (s % 2)) * 512
                    zifo = zt[:, c0:c0 + 3]
                    zg = zt[:, c0 + 3:c0 + 4]
                    nc.tensor.matmul(zifo, lhsT=ident,
                                     rhs=zx_ifo[:, se, :],
                                     start=True, stop=(s == 0),
                                     skip_group_check=True)
                    if s > 0:
                        for g in range(3):
                            nc.tensor.matmul(
                                zt[:, c0 + g:c0 + g + 1], lhsT=bdr(d, g),
                                rhs=hT[d][:], start=False, stop=(g == 2),
                                skip_group_check=True)
                        nc.tensor.matmul(zg, lhsT=bdr(d, 3), rhs=hT[d][:],
                                         start=True, stop=True)
                        tg_b = zx_g[:, se:se + 1]
                    else:
                        tg_b = None
                    sg = gp.tile([128, 3], BF, tag=f"sg{d}")
                    tgl = gp.tile([128, 1], BF, tag=f"tg{d}")
                    tcl = gp.tile([128, 1], BF, tag=f"tc{d}")
                    tm1 = gp.tile([128, 1], FP, tag=f"tm1{d}")
                    nc.scalar.activation(sg[:], zifo, AF.Sigmoid)
                    if s > 0:
                        nc.scalar.activation(tgl[:], zg, AF.Tanh, bias=tg_b)
                    else:
                        nc.scalar.activation(tgl[:], zx_g[:, se:se + 1],
                                             AF.Tanh)
                    # tm1 = sig_i * tanh_g   (tgl as per-partition scalar)
                    nc.vector.scalar_tensor_tensor(
                        tm1[:], sg[:, 0:1], tgl[:], sg[:, 0:1],
                        ALU.mult, ALU.bypass)
                    if s > 0:
                        # c = sig_f * c + tm1  (cT as per-partition scalar)
                        nc.vector.scalar_tensor_tensor(
                            cT[d][:], sg[:, 1:2], cT[d][:], tm1[:],
                            ALU.mult, ALU.add)
                    else:
                        nc.vector.tensor_copy(cT[d][:], tm1[:])
                    nc.scalar.activation(tcl[:], cT[d][:], AF.Tanh)
                    # h = sig_o * tanh_c
                    nc.vector.scalar_tensor_tensor(
                        hT[d][:], sg[:, 2:3], tcl[:], sg[:, 2:3],
                        ALU.mult, ALU.bypass)

            # ---------------- dense + sigmoid ----------------
            fo = zt[0:BL, 2044:2045]
            nc.tensor.matmul(fo, lhsT=wdx[0], rhs=hT[0][:],
                             start=True, stop=False, skip_group_check=True)
            nc.tensor.matmul(fo, lhsT=wdx[1], rhs=hT[1][:],
                             start=False, stop=True, skip_group_check=True)
            res = gp.tile([BL, 1], FP, tag="res")
            nc.scalar.activation(res[:], fo, AF.Sigmoid, bias=bd)
            nc.sync.dma_start(out=out[:], in_=res[:])

    nc.compile()
    return nc


def _prep_inputs(x, k_conv, r_conv, b_conv, k_f, r_f, b_f, k_b, r_b, b_b,
                 w_d, b_d):
    """Host-side: gate reorder, block-diag expansion, x transpose."""
    assert np.all(b_conv == 0.0), "nonzero b_conv not supported by this kernel"
    k_conv = _reorder_gates(np.asarray(k_conv, np.float32), F)
    r_conv = _reorder_gates(np.asarray(r_conv, np.float32), F)
    k_f = _reorder_gates(np.asarray(k_f, np.float32), U)
    r_f = _reorder_gates(np.asarray(r_f, np.float32), U)
    b_f = _reorder_gates(np.asarray(b_f, np.float32), U)
    k_b = _reorder_gates(np.asarray(k_b, np.float32), U)
    r_b = _reorder_gates(np.asarray(r_b, np.float32), U)
    b_b = _reorder_gates(np.asarray(b_b, np.float32), U)

    import ml_dtypes
    w_bf = np.zeros((128, WBF_COLS), np.float32)
    w_all = np.zeros((128, W_COLS), np.float32)
    for g in range(4):
        for tap in range(2):
            wi = np.zeros((128, 128), np.float32)
            wr = np.zeros((128, 128), np.float32)
            for b in range(4):
                sl = slice(b * 32, (b + 1) * 32)
                wi[sl, sl] = k_conv[tap, :, g * 32:(g + 1) * 32]
                wr[sl, sl] = r_conv[tap, :, g * 32:(g + 1) * 32]
            w_bf[:, (g * 2 + tap) * 128:(g * 2 + tap + 1) * 128] = wi
            w_bf[:, (8 + g * 2 + tap) * 128:(9 + g * 2 + tap) * 128] = wr
    w_bf[:, 2048:2176] = np.eye(128, dtype=np.float32)
    w_d = np.asarray(w_d, np.float32)
    for d, (kk, rr, bb) in enumerate([(k_f, r_f, b_f), (k_b, r_b, b_b)]):
        for g in range(4):
            bk = np.zeros((128, 128), np.float32)
            br = np.zeros((128, 128), np.float32)
            for b in range(4):
                sl = slice(b * 32, (b + 1) * 32)
                bk[sl, sl] = kk[:, g * 32:(g + 1) * 32]
                br[sl, sl] = rr[:, g * 32:(g + 1) * 32]
            w_bf[:, 2176 + (d * 4 + g) * 128:2304 + (d * 4 + g) * 128] = bk
            w_bf[:, 3200 + (d * 4 + g) * 128:3328 + (d * 4 + g) * 128] = br
            w_all[:, d * 4 + g] = np.tile(bb[g * 32:(g + 1) * 32], 4)
        wx = np.zeros((128, 4), np.float32)
        for b in range(4):
            wx[b * 32:(b + 1) * 32, b] = w_d[d * 32:(d + 1) * 32, 0]
        w_bf[:, 4224 + d * 4:4228 + d * 4] = wx
    w_all[:, 8] = 0.5
    w_all[0:4, 9] = np.float32(np.asarray(b_d).reshape(-1)[0])
    w_bf = w_bf.astype(ml_dtypes.bfloat16)

    # x (B,T,512,C) -> per-core (128=(b,c), T, (tap,j)): x2[b*32+c, t, tap*256+j]
    #   = x[b, t, 2j+tap, c]
    x = np.asarray(x, np.float32).reshape(B, T, LO, 2, C)
    # -> (B, C, T, tap, j)
    xt = np.ascontiguousarray(x.transpose(0, 4, 1, 3, 2))
    x2_full = xt.reshape(B * C, T, 2 * LO)

    x2_full = x2_full.astype(ml_dtypes.bfloat16)
    in_maps = []
    for core in range(NCORES):
        x2c = np.ascontiguousarray(
            x2_full[core * BL * C:(core + 1) * BL * C])
        in_maps.append({"x2": x2c, "w_bf": w_bf, "w_all": w_all})
    return in_maps


def kernel(**inputs) -> np.ndarray:
    if "nc" not in _CACHE:
        _CACHE["nc"] = _build_graph()
    nc = _CACHE["nc"]
    in_maps = _prep_inputs(**inputs)
    res = run_bass_kernel_spmd(nc, in_maps, core_ids=list(range(NCORES)))
    outs = [res.results[i]["out"].reshape(BL, 1) for i in range(NCORES)]
    return np.concatenate(outs, axis=0).astype(np.float32)


# revision 21
# speedup vs baseline: 1.0447x; 1.0420x over previous
"""Trainium2 Bass kernel: ConvLSTM1D -> BiLSTM -> dense sigmoid.

Reference model (per full batch B=32):
  h = ConvLSTM1D(x (B,64,512,32); k (2,32,128) stride2, r (2,32,128), hard_sigmoid)
      -> final hidden (B, 256, 32)
  hf = LSTM(h) last state; hb = LSTM(h reversed) last state  (U=32 each)
  out = sigmoid(concat(hf,hb) @ w_d + b_d)   (B, 1)

Sharding: pure data parallelism, batch 32 -> 8 cores x 4.

Per-core layout choices:
  ConvLSTM scan state/gates: partitions = (b4, ch32) = 128, free = j (256).
    Matmuls use block-diagonal weights lhsT[(b',cin),(b,ch)] = delta_bb' W[cin,ch]
    (K=128, M=128, N=256, float32r -> 1 cycle/row) accumulating input-conv taps
    and recurrent-conv taps into one PSUM group per gate.
  BiLSTM: transposed layout, partitions = (gate,U) = 128, free = batch (4).
    Two interleaved chains (fwd, bwd); zx injected by identity-matmul.
Gate order is host-reordered from Keras (i,f,g,o) to (i,f,o,g) so the three
hard-sigmoid/sigmoid gates are contiguous.
"""

import numpy as np

import concourse.bass as bass
import concourse.bacc as bacc
import concourse.mybir as mybir
from concourse.tile import TileContext
from concourse.bass_utils import run_bass_kernel_spmd

B, T, L, C = 32, 64, 512, 32
F = 32          # conv filters
U = 32          # lstm units
NCORES = 8
BL = B // NCORES          # 4 local batch
LO = L // 2               # 256 spatial after stride-2 conv
G4 = 4 * F                # 128 gate channels

FP = mybir.dt.float32
BF = mybir.dt.bfloat16

# w_bf column layout (bf16): big matmul weights
#  [0:2048)    16 block-diag (128x128) conv weights, index (g*2+tap)*128,
#              first 8 = input conv, next 8 = recurrent conv
#  [2048:2176) identity 128x128
#  [2176:3200) 8 block-diag zx weights bdk[d][g][(b,ch),(b,U)]
#  [3200:4224) 8 block-diag lstm rec weights bdr[d][g][(b,U'),(b,U)]
#  [4224:4232) dense wdx[d] (128,4): [(b,u), b] = delta * w_d[u+32d]
WBF_COLS = 4232
# w_all column layout (f32): biases
#  [0:8)       lstm biases per (d,g): (128,1) = b_d[g*32+u]
#  [8]         0.5 constant
#  [9]         b_d (dense bias) replicated
W_COLS = 10

_CACHE = {}


def _reorder_gates(w, n):
    # last dim (4n): keras order i,f,g,o -> i,f,o,g
    i, f, g, o = np.split(w, 4, axis=-1)
    return np.concatenate([i, f, o, g], axis=-1)


def _build_graph():
    nc = bacc.Bacc("TRN2")
    x2 = nc.declare_dram_parameter("x2", [128, T, 2 * LO], BF, isOutput=False)
    w_bf = nc.declare_dram_parameter("w_bf", [128, WBF_COLS], BF, isOutput=False)
    w_all = nc.declare_dram_parameter("w_all", [128, W_COLS], FP, isOutput=False)
    out = nc.declare_dram_parameter("out", [BL, 1], FP, isOutput=True)

    AF = mybir.ActivationFunctionType
    ALU = mybir.AluOpType

    with TileContext(nc) as tc:
        with (
            tc.tile_pool(name="w", bufs=1) as wp,
            tc.tile_pool(name="x", bufs=4) as xp,
            tc.tile_pool(name="st", bufs=1) as sp,
            tc.tile_pool(name="g", bufs=3) as gp,
            tc.tile_pool(name="zp", bufs=2, space="PSUM") as zp,
            tc.tile_pool(name="sc", bufs=1, space="PSUM") as scp,
        ):
            W = wp.tile([128, W_COLS], FP)
            nc.sync.dma_start(out=W[:], in_=w_all[:])
            WB = wp.tile([128, WBF_COLS], BF)
            nc.sync.dma_start(out=WB[:], in_=w_bf[:])

            def wconv(idx):  # (128,128) bf16 block-diag conv weight
                return WB[:, idx * 128:(idx + 1) * 128]

            ident = WB[:, 2048:2176]

            def bdk(d, g):  # zx input weights, block-diag (bf16)
                o = 2176 + (d * 4 + g) * 128
                return WB[:, o:o + 128]

            def bdr(d, g):  # lstm recurrent weights, block-diag (bf16)
                o = 3200 + (d * 4 + g) * 128
                return WB[:, o:o + 128]

            wdx = [WB[:, 4224:4228], WB[:, 4228:4232]]
            bls = [[W[:, d * 4 + g:d * 4 + g + 1] for g in range(4)]
                   for d in range(2)]
            half = W[:, 8:9]
            bd = W[0:4, 9:10]

            # ---------------- Phase A: ConvLSTM scan over T ----------------
            h_sb = sp.tile([128, LO + 1], BF)   # col 256 stays zero (pad)
            c_sb = sp.tile([128, LO], FP)
            nc.vector.memset(h_sb[:, LO:LO + 1], 0.0)

            # two PSUM tiles (one bank each) so gate reads never falsely
            # serialize against later gates' matmul writes:
            # zA = [g | f], zB = [i | o]; emission order g, i, f, o
            for t in range(T):
                xt = xp.tile([128, 2, LO], BF, tag="xt")
                nc.sync.dma_start(out=xt[:], in_=x2[:, t, :])
                zA = zp.tile([128, 2 * LO], FP, tag="zA")
                zB = zp.tile([128, 2 * LO], FP, tag="zB")
                sig = gp.tile([128, 3, LO], BF, tag="sig")
                tg = gp.tile([128, LO], BF, tag="tg")
                tc_t = gp.tile([128, LO], BF, tag="tc")
                tmp = gp.tile([128, LO], BF, tag="tmp")
                c2 = gp.tile([128, LO], FP, tag="c2")

                def convmm(g, zt_, pos):
                    zg = zt_[:, pos * LO:(pos + 1) * LO]
                    n_acc = 4 if t > 0 else 2
                    k = 0
                    for tap in range(2):
                        nc.tensor.matmul(
                            zg, lhsT=wconv(g * 2 + tap), rhs=xt[:, tap, :],
                            start=(k == 0), stop=(k == n_acc - 1))
                        k += 1
                    if t > 0:
                        for tap in range(2):
                            nc.tensor.matmul(
                                zg, lhsT=wconv(8 + g * 2 + tap),
                                rhs=h_sb[:, tap:tap + LO],
                                start=False, stop=(k == n_acc - 1))
                            k += 1

                # gate index in weights: 0=i 1=f 2=o 3=g (host order i,f,o,g)
                convmm(3, zA, 0)
                nc.scalar.activation(tg[:], zA[:, 0:LO], AF.Tanh)
                convmm(0, zB, 0)
                nc.scalar.activation(sig[:, 0, :], zB[:, 0:LO],
                                     AF.Relu, bias=half, scale=0.2)
                # tmp = min(sig_i,1) * tanh(zg)
                nc.vector.scalar_tensor_tensor(
                    (c_sb[:] if t == 0 else tmp[:]),
                    sig[:, 0, :], 1.0, tg[:], ALU.min, ALU.mult)
                convmm(1, zA, 1)
                nc.scalar.activation(sig[:, 1, :], zA[:, LO:2 * LO],
                                     AF.Relu, bias=half, scale=0.2)
                if t > 0:
                    nc.vector.scalar_tensor_tensor(
                        c2[:], sig[:, 1, :], 1.0, c_sb[:], ALU.min, ALU.mult)
                    nc.vector.tensor_tensor(c_sb[:], tmp[:], c2[:], ALU.add)
                convmm(2, zB, 1)
                nc.scalar.activation(sig[:, 2, :], zB[:, LO:2 * LO],
                                     AF.Relu, bias=half, scale=0.2)
                nc.scalar.activation(tc_t[:], c_sb[:], AF.Tanh)
                nc.vector.scalar_tensor_tensor(
                    h_sb[:, 0:LO], sig[:, 2, :], 1.0, tc_t[:],
                    ALU.min, ALU.mult)

            # ---------------- Phase B: bidirectional LSTM over LO ----------
            # Layout: partitions = (b,U) = 128, free = gate cols. No partition
            # shifts anywhere (walrus verifier requires same partitions).
            # zx[d][g] (128, LO): input-side gates + lstm bias, injected into
            # the per-step PSUM via identity matmul (i,f,o) / ACT bias (g).
            zxs = []
            for d in range(2):
                psA = zp.tile([128, 2 * LO], FP, tag="zA", name=f"zxpsA{d}")
                psB = zp.tile([128, 2 * LO], FP, tag="zB", name=f"zxpsB{d}")
                pss = [psA, psB]

                def ps_slice(g):
                    return pss[g // 2][:, (g % 2) * LO:(g % 2 + 1) * LO]

                for g in range(4):
                    nc.tensor.matmul(
                        ps_slice(g), lhsT=bdk(d, g),
                        rhs=h_sb[:, 0:LO],
                        start=True, stop=True)
                zx_ifo = sp.tile([128, LO, 3], BF, tag=f"zxifo{d}",
                                 name=f"zxifo{d}")
                zx_g = sp.tile([128, LO], FP, tag=f"zxg{d}", name=f"zxg{d}")
                # evacuation + lstm-bias fold; split across ACT and DVE
                nc.scalar.activation(
                    zx_ifo[:, :, 0], ps_slice(0), AF.Identity, bias=bls[d][0])
                nc.vector.scalar_tensor_tensor(
                    zx_ifo[:, :, 1], ps_slice(1), bls[d][1],
                    h_sb[:, 0:LO], ALU.add, ALU.bypass)
                nc.scalar.activation(
                    zx_ifo[:, :, 2], ps_slice(2), AF.Identity,
                    bias=bls[d][2])
                nc.vector.scalar_tensor_tensor(
                    zx_g[:], ps_slice(3), bls[d][3],
                    h_sb[:, 0:LO], ALU.add, ALU.bypass)
                zxs.append((zx_ifo, zx_g))

            # state: hT[d] bf16 (feeds bf16 matmul), cT[d] f32
            hT = [sp.tile([128, 1], BF, tag=f"hT{d}", name=f"hT{d}")
                  for d in range(2)]
            cT = [sp.tile([128, 1], FP, tag=f"cT{d}", name=f"cT{d}")
                  for d in range(2)]
            # 4 PSUM banks: region per (dir, parity) so PE writes of one
            # step never share a bank with ACT reads of the previous step
            zt = scp.tile([128, 2048], FP, tag="zt")

            def pb_mm(s, d):
                se = s if d == 0 else LO - 1 - s
                zx_ifo, _ = zxs[d]
                c0 = (d * 2 + (s % 2)) * 512
                zifo = zt[:, c0:c0 + 3]
                zg = zt[:, c0 + 3:c0 + 4]
                # g-gate rec first so tanh_g's input is ready early
                if s > 0:
                    nc.tensor.matmul(zg, lhsT=bdr(d, 3), rhs=hT[d][:],
                                     start=True, stop=True,
                                     skip_group_check=True)
                nc.tensor.matmul(zifo, lhsT=ident,
                                 rhs=zx_ifo[:, se, :],
                                 start=True, stop=(s == 0),
                                 skip_group_check=True)
                if s > 0:
                    for g in range(3):
                        nc.tensor.matmul(
                            zt[:, c0 + g:c0 + g + 1], lhsT=bdr(d, g),
                            rhs=hT[d][:], start=False, stop=(g == 2),
                            skip_group_check=True)
                return zifo, zg, se

            tiles = {}
            for d in range(2):
                tiles[d] = None

            for s in range(LO):
                zz = [pb_mm(s, 0), pb_mm(s, 1)]
                tl = []
                for d in range(2):
                    tl.append((gp.tile([128, 3], BF, tag=f"sg{d}",
                                       name=f"sg{d}"),
                               gp.tile([128, 1], BF, tag=f"tg{d}",
                                       name=f"tg{d}"),
                               gp.tile([128, 1], BF, tag=f"tc{d}",
                                       name=f"tc{d}"),
                               gp.tile([128, 1], FP, tag=f"tm1{d}",
                                       name=f"tm1{d}")))
                # interleave the two chains op-by-op on each engine
                for d in range(2):
                    zifo, zg, se = zz[d]
                    sg, tgl, tcl, tm1 = tl[d]
                    zx_g = zxs[d][1]
                    if s > 0:
                        nc.scalar.activation(tgl[:], zg, AF.Tanh,
                                             bias=zx_g[:, se:se + 1])
                    else:
                        nc.scalar.activation(tgl[:], zx_g[:, se:se + 1],
                                             AF.Tanh)
                    nc.scalar.activation(sg[:], zifo, AF.Sigmoid)
                    # tm1 = sig_i * tanh_g
                    nc.vector.scalar_tensor_tensor(
                        tm1[:], sg[:, 0:1], tgl[:], sg[:, 0:1],
                        ALU.mult, ALU.bypass)
                    if s > 0:
                        nc.vector.scalar_tensor_tensor(
                            cT[d][:], sg[:, 1:2], cT[d][:], tm1[:],
                            ALU.mult, ALU.add)
                    else:
                        nc.vector.tensor_copy(cT[d][:], tm1[:])
                for d in range(2):
                    sg, tgl, tcl, tm1 = tl[d]
                    nc.scalar.activation(tcl[:], cT[d][:], AF.Tanh)
                    nc.vector.scalar_tensor_tensor(
                        hT[d][:], sg[:, 2:3], tcl[:], sg[:, 2:3],
                        ALU.mult, ALU.bypass)

            # ---------------- dense + sigmoid ----------------
            fo = zt[0:BL, 2044:2045]
            nc.tensor.matmul(fo, lhsT=wdx[0], rhs=hT[0][:],
                             start=True, stop=False, skip_group_check=True)
            nc.tensor.matmul(fo, lhsT=wdx[1], rhs=hT[1][:],
                             start=False, stop=True, skip_group_check=True)
            res = gp.tile([BL, 1], FP, tag="res")
            nc.scalar.activation(res[:], fo, AF.Sigmoid, bias=bd)
            nc.sync.dma_start(out=out[:], in_=res[:])

    nc.compile()
    return nc


def _prep_inputs(x, k_conv, r_conv, b_conv, k_f, r_f, b_f, k_b, r_b, b_b,
                 w_d, b_d):
    """Host-side: gate reorder, block-diag expansion, x transpose."""
    assert np.all(b_conv == 0.0), "nonzero b_conv not supported by this kernel"
    k_conv = _reorder_gates(np.asarray(k_conv, np.float32), F)
    r_conv = _reorder_gates(np.asarray(r_conv, np.float32), F)
    k_f = _reorder_gates(np.asarray(k_f, np.float32), U)
    r_f = _reorder_gates(np.asarray(r_f, np.float32), U)
    b_f = _reorder_gates(np.asarray(b_f, np.float32), U)
    k_b = _reorder_gates(np.asarray(k_b, np.float32), U)
    r_b = _reorder_gates(np.asarray(r_b, np.float32), U)
    b_b = _reorder_gates(np.asarray(b_b, np.float32), U)

    import ml_dtypes
    w_bf = np.zeros((128, WBF_COLS), np.float32)
    w_all = np.zeros((128, W_COLS), np.float32)
    for g in range(4):
        for tap in range(2):
            wi = np.zeros((128, 128), np.float32)
            wr = np.zeros((128, 128), np.float32)
            for b in range(4):
                sl = slice(b * 32, (b + 1) * 32)
                wi[sl, sl] = k_conv[tap, :, g * 32:(g + 1) * 32]
                wr[sl, sl] = r_conv[tap, :, g * 32:(g + 1) * 32]
            w_bf[:, (g * 2 + tap) * 128:(g * 2 + tap + 1) * 128] = wi
            w_bf[:, (8 + g * 2 + tap) * 128:(9 + g * 2 + tap) * 128] = wr
    w_bf[:, 2048:2176] = np.eye(128, dtype=np.float32)
    w_d = np.asarray(w_d, np.float32)
    for d, (kk, rr, bb) in enumerate([(k_f, r_f, b_f), (k_b, r_b, b_b)]):
        for g in range(4):
            bk = np.zeros((128, 128), np.float32)
            br = np.zeros((128, 128), np.float32)
            for b in range(4):
                sl = slice(b * 32, (b + 1) * 32)
                bk[sl, sl] = kk[:, g * 32:(g + 1) * 32]
                br[sl, sl] = rr[:, g * 32:(g + 1) * 32]
            w_bf[:, 2176 + (d * 4 + g) * 128:2304 + (d * 4 + g) * 128] = bk
            w_bf[:, 3200 + (d * 4 + g) * 128:3328 + (d * 4 + g) * 128] = br
            w_all[:, d * 4 + g] = np.tile(bb[g * 32:(g + 1) * 32], 4)
        wx = np.zeros((128, 4), np.float32)
        for b in range(4):
            wx[b * 32:(b + 1) * 32, b] = w_d[d * 32:(d + 1) * 32, 0]
        w_bf[:, 4224 + d * 4:4228 + d * 4] = wx
    w_all[:, 8] = 0.5
    w_all[0:4, 9] = np.float32(np.asarray(b_d).reshape(-1)[0])
    w_bf = w_bf.astype(ml_dtypes.bfloat16)

    # x (B,T,512,C) -> per-core (128=(b,c), T, (tap,j)): x2[b*32+c, t, tap*256+j]
    #   = x[b, t, 2j+tap, c]
    x = np.asarray(x, np.float32).reshape(B, T, LO, 2, C)
    # -> (B, C, T, tap, j)
    xt = np.ascontiguousarray(x.transpose(0, 4, 1, 3, 2))
    x2_full = xt.reshape(B * C, T, 2 * LO)

    x2_full = x2_full.astype(ml_dtypes.bfloat16)
    in_maps = []
    for core in range(NCORES):
        x2c = np.ascontiguousarray(
            x2_full[core * BL * C:(core + 1) * BL * C])
        in_maps.append({"x2": x2c, "w_bf": w_bf, "w_all": w_all})
    return in_maps


def kernel(**inputs) -> np.ndarray:
    if "nc" not in _CACHE:
        _CACHE["nc"] = _build_graph()
    nc = _CACHE["nc"]
    in_maps = _prep_inputs(**inputs)
    res = run_bass_kernel_spmd(nc, in_maps, core_ids=list(range(NCORES)))
    outs = [res.results[i]["out"].reshape(BL, 1) for i in range(NCORES)]
    return np.concatenate(outs, axis=0).astype(np.float32)


# revision 22
# speedup vs baseline: 1.2542x; 1.2005x over previous
"""Trainium2 Bass kernel: ConvLSTM1D -> BiLSTM -> dense sigmoid.

Reference model (per full batch B=32):
  h = ConvLSTM1D(x (B,64,512,32); k (2,32,128) stride2, r (2,32,128), hard_sigmoid)
      -> final hidden (B, 256, 32)
  hf = LSTM(h) last state; hb = LSTM(h reversed) last state  (U=32 each)
  out = sigmoid(concat(hf,hb) @ w_d + b_d)   (B, 1)

Sharding: pure data parallelism, batch 32 -> 8 cores x 4.

Per-core layout choices:
  ConvLSTM scan state/gates: partitions = (b4, ch32) = 128, free = j (256).
    Matmuls use block-diagonal weights lhsT[(b',cin),(b,ch)] = delta_bb' W[cin,ch]
    (K=128, M=128, N=256, float32r -> 1 cycle/row) accumulating input-conv taps
    and recurrent-conv taps into one PSUM group per gate.
  BiLSTM: transposed layout, partitions = (gate,U) = 128, free = batch (4).
    Two interleaved chains (fwd, bwd); zx injected by identity-matmul.
Gate order is host-reordered from Keras (i,f,g,o) to (i,f,o,g) so the three
hard-sigmoid/sigmoid gates are contiguous.
"""

import numpy as np

import concourse.bass as bass
import concourse.bacc as bacc
import concourse.mybir as mybir
from concourse.tile import TileContext
from concourse.bass_utils import run_bass_kernel_spmd

B, T, L, C = 32, 64, 512, 32
F = 32          # conv filters
U = 32          # lstm units
NCORES = 8
BL = B // NCORES          # 4 local batch
LO = L // 2               # 256 spatial after stride-2 conv
G4 = 4 * F                # 128 gate channels

FP = mybir.dt.float32
BF = mybir.dt.bfloat16

# w_bf column layout (bf16): big matmul weights
#  [0:2048)    16 block-diag (128x128) conv weights, index (g*2+tap)*128,
#              first 8 = input conv, next 8 = recurrent conv
#  [2048:2176) identity 128x128
#  [2176:3200) 8 block-diag zx weights bdk[d][g][(b,ch),(b,U)]
#  [3200:4224) 8 block-diag lstm rec weights bdr[d][g][(b,U'),(b,U)]
#  [4224:4232) dense wdx[d] (128,4): [(b,u), b] = delta * w_d[u+32d]
WBF_COLS = 4232
# w_all column layout (f32): biases
#  [0:8)       lstm biases per (d,g): (128,1) = b_d[g*32+u]
#  [8]         0.5 constant
#  [9]         b_d (dense bias) replicated
W_COLS = 10

_CACHE = {}


def _reorder_gates(w, n):
    # last dim (4n): keras order i,f,g,o -> i,f,o,g
    i, f, g, o = np.split(w, 4, axis=-1)
    return np.concatenate([i, f, o, g], axis=-1)


def _build_graph():
    nc = bacc.Bacc("TRN2")
    x2 = nc.declare_dram_parameter("x2", [128, T, 2 * LO], BF, isOutput=False)
    w_bf = nc.declare_dram_parameter("w_bf", [128, WBF_COLS], BF, isOutput=False)
    w_all = nc.declare_dram_parameter("w_all", [128, W_COLS], FP, isOutput=False)
    out = nc.declare_dram_parameter("out", [BL, 1], FP, isOutput=True)

    AF = mybir.ActivationFunctionType
    ALU = mybir.AluOpType

    with TileContext(nc) as tc:
        with (
            tc.tile_pool(name="w", bufs=1) as wp,
            tc.tile_pool(name="x", bufs=4) as xp,
            tc.tile_pool(name="st", bufs=1) as sp,
            tc.tile_pool(name="g", bufs=3) as gp,
            tc.tile_pool(name="zp", bufs=4, space="PSUM") as zp,
        ):
            W = wp.tile([128, W_COLS], FP)
            nc.sync.dma_start(out=W[:], in_=w_all[:])
            WB = wp.tile([128, WBF_COLS], BF)
            nc.sync.dma_start(out=WB[:], in_=w_bf[:])

            def wconv(idx):  # (128,128) bf16 block-diag conv weight
                return WB[:, idx * 128:(idx + 1) * 128]

            ident = WB[:, 2048:2176]

            def bdk(d, g):  # zx input weights, block-diag (bf16)
                o = 2176 + (d * 4 + g) * 128
                return WB[:, o:o + 128]

            def bdr(d, g):  # lstm recurrent weights, block-diag (bf16)
                o = 3200 + (d * 4 + g) * 128
                return WB[:, o:o + 128]

            wdx = [WB[:, 4224:4228], WB[:, 4228:4232]]
            bls = [[W[:, d * 4 + g:d * 4 + g + 1] for g in range(4)]
                   for d in range(2)]
            half = W[:, 8:9]
            bd = W[0:4, 9:10]

            # ---------------- Phase A: ConvLSTM scan over T ----------------
            h_sb = sp.tile([128, LO + 1], BF)   # col 256 stays zero (pad)
            c_sb = sp.tile([128, LO], FP)
            nc.vector.memset(h_sb[:, LO:LO + 1], 0.0)

            # two PSUM tiles (one bank each) so gate reads never falsely
            # serialize against later gates' matmul writes:
            # zA = [g | f], zB = [i | o]; emission order g, i, f, o
            for t in range(T):
                xt = xp.tile([128, 2, LO], BF, tag="xt")
                nc.sync.dma_start(out=xt[:], in_=x2[:, t, :])
                zA = zp.tile([128, 2 * LO], FP, tag="zA")
                zB = zp.tile([128, 2 * LO], FP, tag="zB")
                sig = gp.tile([128, 3, LO], BF, tag="sig")
                tg = gp.tile([128, LO], BF, tag="tg")
                tc_t = gp.tile([128, LO], BF, tag="tc")
                tmp = gp.tile([128, LO], BF, tag="tmp")
                c2 = gp.tile([128, LO], FP, tag="c2")

                def convmm(g, zt_, pos):
                    zg = zt_[:, pos * LO:(pos + 1) * LO]
                    n_acc = 4 if t > 0 else 2
                    k = 0
                    for tap in range(2):
                        nc.tensor.matmul(
                            zg, lhsT=wconv(g * 2 + tap), rhs=xt[:, tap, :],
                            start=(k == 0), stop=(k == n_acc - 1))
                        k += 1
                    if t > 0:
                        for tap in range(2):
                            nc.tensor.matmul(
                                zg, lhsT=wconv(8 + g * 2 + tap),
                                rhs=h_sb[:, tap:tap + LO],
                                start=False, stop=(k == n_acc - 1))
                            k += 1

                # gate index in weights: 0=i 1=f 2=o 3=g (host order i,f,o,g)
                convmm(3, zA, 0)
                nc.scalar.activation(tg[:], zA[:, 0:LO], AF.Tanh)
                convmm(0, zB, 0)
                nc.scalar.activation(sig[:, 0, :], zB[:, 0:LO],
                                     AF.Relu, bias=half, scale=0.2)
                # tmp = min(sig_i,1) * tanh(zg)
                nc.vector.scalar_tensor_tensor(
                    (c_sb[:] if t == 0 else tmp[:]),
                    sig[:, 0, :], 1.0, tg[:], ALU.min, ALU.mult)
                convmm(1, zA, 1)
                nc.scalar.activation(sig[:, 1, :], zA[:, LO:2 * LO],
                                     AF.Relu, bias=half, scale=0.2)
                if t > 0:
                    nc.vector.scalar_tensor_tensor(
                        c2[:], sig[:, 1, :], 1.0, c_sb[:], ALU.min, ALU.mult)
                    nc.vector.tensor_tensor(c_sb[:], tmp[:], c2[:], ALU.add)
                convmm(2, zB, 1)
                nc.scalar.activation(sig[:, 2, :], zB[:, LO:2 * LO],
                                     AF.Relu, bias=half, scale=0.2)
                nc.scalar.activation(tc_t[:], c_sb[:], AF.Tanh)
                nc.vector.scalar_tensor_tensor(
                    h_sb[:, 0:LO], sig[:, 2, :], 1.0, tc_t[:],
                    ALU.min, ALU.mult)

            # ---------------- Phase B: bidirectional LSTM over LO ----------
            # Layout: partitions = (b,U) = 128, free = gate cols. No partition
            # shifts anywhere (walrus verifier requires same partitions).
            # zx[d][g] (128, LO): input-side gates + lstm bias, injected into
            # the per-step PSUM via identity matmul (i,f,o) / ACT bias (g).
            zxs = []
            for d in range(2):
                psA = zp.tile([128, 2 * LO], FP, tag="zA", name=f"zxpsA{d}")
                psB = zp.tile([128, 2 * LO], FP, tag="zB", name=f"zxpsB{d}")
                pss = [psA, psB]

                def ps_slice(g):
                    return pss[g // 2][:, (g % 2) * LO:(g % 2 + 1) * LO]

                for g in range(4):
                    nc.tensor.matmul(
                        ps_slice(g), lhsT=bdk(d, g),
                        rhs=h_sb[:, 0:LO],
                        start=True, stop=True)
                zx_ifo = sp.tile([128, LO, 3], BF, tag=f"zxifo{d}",
                                 name=f"zxifo{d}")
                zx_g = sp.tile([128, LO], FP, tag=f"zxg{d}", name=f"zxg{d}")
                # evacuation + lstm-bias fold; split across ACT and DVE
                nc.scalar.activation(
                    zx_ifo[:, :, 0], ps_slice(0), AF.Identity, bias=bls[d][0])
                nc.vector.scalar_tensor_tensor(
                    zx_ifo[:, :, 1], ps_slice(1), bls[d][1],
                    h_sb[:, 0:LO], ALU.add, ALU.bypass)
                nc.scalar.activation(
                    zx_ifo[:, :, 2], ps_slice(2), AF.Identity,
                    bias=bls[d][2])
                nc.vector.scalar_tensor_tensor(
                    zx_g[:], ps_slice(3), bls[d][3],
                    h_sb[:, 0:LO], ALU.add, ALU.bypass)
                zxs.append((zx_ifo, zx_g))

            # state: hT[d] bf16 (feeds bf16 matmul), cT[d] f32
            hT = [sp.tile([128, 1], BF, tag=f"hT{d}", name=f"hT{d}")
                  for d in range(2)]
            cT = [sp.tile([128, 1], FP, tag=f"cT{d}", name=f"cT{d}")
                  for d in range(2)]

            def pb_mm(s, d):
                se = s if d == 0 else LO - 1 - s
                zx_ifo, _ = zxs[d]
                # fresh PSUM slots; zifo and zg in different banks (zA/zB)
                zifo = zp.tile([128, 2 * LO], FP, tag="zA",
                               name=f"zi{d}")[:, 0:3]
                zg = zp.tile([128, 2 * LO], FP, tag="zB",
                             name=f"zgt{d}")[:, 0:1]
                # inject first: it has no dependency on h, runs ahead
                nc.tensor.matmul(zifo, lhsT=ident,
                                 rhs=zx_ifo[:, se, :],
                                 start=True, stop=(s == 0),
                                 skip_group_check=True)
                if s > 0:
                    nc.tensor.matmul(zg, lhsT=bdr(d, 3), rhs=hT[d][:],
                                     start=True, stop=True,
                                     skip_group_check=True)
                    for g in range(3):
                        nc.tensor.matmul(
                            zifo[:, g:g + 1], lhsT=bdr(d, g),
                            rhs=hT[d][:], start=False, stop=(g == 2),
                            skip_group_check=True)
                return zifo, zg, se

            for s in range(LO):
                zz = [pb_mm(s, 0), pb_mm(s, 1)]
                tl = []
                for d in range(2):
                    tl.append((gp.tile([128, 3], BF, tag=f"sg{d}",
                                       name=f"sg{d}"),
                               gp.tile([128, 1], BF, tag=f"tg{d}",
                                       name=f"tg{d}"),
                               gp.tile([128, 1], BF, tag=f"tc{d}",
                                       name=f"tc{d}"),
                               gp.tile([128, 1], FP, tag=f"tm1{d}",
                                       name=f"tm1{d}")))
                # interleave the two chains op-by-op on each engine
                for d in range(2):
                    zifo, zg, se = zz[d]
                    sg, tgl, tcl, tm1 = tl[d]
                    zx_g = zxs[d][1]
                    nc.scalar.activation(sg[:], zifo, AF.Sigmoid)
                    if s > 0:
                        nc.scalar.activation(tgl[:], zg, AF.Tanh,
                                             bias=zx_g[:, se:se + 1])
                    else:
                        nc.scalar.activation(tgl[:], zx_g[:, se:se + 1],
                                             AF.Tanh)
                    # tm1 = sig_i * tanh_g
                    nc.vector.scalar_tensor_tensor(
                        tm1[:], sg[:, 0:1], tgl[:], sg[:, 0:1],
                        ALU.mult, ALU.bypass)
                    if s > 0:
                        nc.vector.scalar_tensor_tensor(
                            cT[d][:], sg[:, 1:2], cT[d][:], tm1[:],
                            ALU.mult, ALU.add)
                    else:
                        nc.vector.tensor_copy(cT[d][:], tm1[:])
                for d in range(2):
                    sg, tgl, tcl, tm1 = tl[d]
                    nc.scalar.activation(tcl[:], cT[d][:], AF.Tanh)
                    nc.vector.scalar_tensor_tensor(
                        hT[d][:], sg[:, 2:3], tcl[:], sg[:, 2:3],
                        ALU.mult, ALU.bypass)

            # ---------------- dense + sigmoid ----------------
            fo = zp.tile([128, 2 * LO], FP, tag="zA",
                         name="fo")[0:BL, 0:1]
            nc.tensor.matmul(fo, lhsT=wdx[0], rhs=hT[0][:],
                             start=True, stop=False, skip_group_check=True)
            nc.tensor.matmul(fo, lhsT=wdx[1], rhs=hT[1][:],
                             start=False, stop=True, skip_group_check=True)
            res = gp.tile([BL, 1], FP, tag="res")
            nc.scalar.activation(res[:], fo, AF.Sigmoid, bias=bd)
            nc.sync.dma_start(out=out[:], in_=res[:])

    nc.compile()
    return nc


def _prep_inputs(x, k_conv, r_conv, b_conv, k_f, r_f, b_f, k_b, r_b, b_b,
                 w_d, b_d):
    """Host-side: gate reorder, block-diag expansion, x transpose."""
    assert np.all(b_conv == 0.0), "nonzero b_conv not supported by this kernel"
    k_conv = _reorder_gates(np.asarray(k_conv, np.float32), F)
    r_conv = _reorder_gates(np.asarray(r_conv, np.float32), F)
    k_f = _reorder_gates(np.asarray(k_f, np.float32), U)
    r_f = _reorder_gates(np.asarray(r_f, np.float32), U)
    b_f = _reorder_gates(np.asarray(b_f, np.float32), U)
    k_b = _reorder_gates(np.asarray(k_b, np.float32), U)
    r_b = _reorder_gates(np.asarray(r_b, np.float32), U)
    b_b = _reorder_gates(np.asarray(b_b, np.float32), U)

    import ml_dtypes
    w_bf = np.zeros((128, WBF_COLS), np.float32)
    w_all = np.zeros((128, W_COLS), np.float32)
    for g in range(4):
        for tap in range(2):
            wi = np.zeros((128, 128), np.float32)
            wr = np.zeros((128, 128), np.float32)
            for b in range(4):
                sl = slice(b * 32, (b + 1) * 32)
                wi[sl, sl] = k_conv[tap, :, g * 32:(g + 1) * 32]
                wr[sl, sl] = r_conv[tap, :, g * 32:(g + 1) * 32]
            w_bf[:, (g * 2 + tap) * 128:(g * 2 + tap + 1) * 128] = wi
            w_bf[:, (8 + g * 2 + tap) * 128:(9 + g * 2 + tap) * 128] = wr
    w_bf[:, 2048:2176] = np.eye(128, dtype=np.float32)
    w_d = np.asarray(w_d, np.float32)
    for d, (kk, rr, bb) in enumerate([(k_f, r_f, b_f), (k_b, r_b, b_b)]):
        for g in range(4):
            bk = np.zeros((128, 128), np.float32)
            br = np.zeros((128, 128), np.float32)
            for b in range(4):
                sl = slice(b * 32, (b + 1) * 32)
                bk[sl, sl] = kk[:, g * 32:(g + 1) * 32]
                br[sl, sl] = rr[:, g * 32:(g + 1) * 32]
            w_bf[:, 2176 + (d * 4 + g) * 128:2304 + (d * 4 + g) * 128] = bk
            w_bf[:, 3200 + (d * 4 + g) * 128:3328 + (d * 4 + g) * 128] = br
            w_all[:, d * 4 + g] = np.tile(bb[g * 32:(g + 1) * 32], 4)
        wx = np.zeros((128, 4), np.float32)
        for b in range(4):
            wx[b * 32:(b + 1) * 32, b] = w_d[d * 32:(d + 1) * 32, 0]
        w_bf[:, 4224 + d * 4:4228 + d * 4] = wx
    w_all[:, 8] = 0.5
    w_all[0:4, 9] = np.float32(np.asarray(b_d).reshape(-1)[0])
    w_bf = w_bf.astype(ml_dtypes.bfloat16)

    # x (B,T,512,C) -> per-core (128=(b,c), T, (tap,j)): x2[b*32+c, t, tap*256+j]
    #   = x[b, t, 2j+tap, c]
    x = np.asarray(x, np.float32).reshape(B, T, LO, 2, C)
    # -> (B, C, T, tap, j)
    xt = np.ascontiguousarray(x.transpose(0, 4, 1, 3, 2))
    x2_full = xt.reshape(B * C, T, 2 * LO)

    x2_full = x2_full.astype(ml_dtypes.bfloat16)
    in_maps = []
    for core in range(NCORES):
        x2c = np.ascontiguousarray(
            x2_full[core * BL * C:(core + 1) * BL * C])
        in_maps.append({"x2": x2c, "w_bf": w_bf, "w_all": w_all})
    return in_maps


def kernel(**inputs) -> np.ndarray:
    if "nc" not in _CACHE:
        _CACHE["nc"] = _build_graph()
    nc = _CACHE["nc"]
    in_maps = _prep_inputs(**inputs)
    res = run_bass_kernel_spmd(nc, in_maps, core_ids=list(range(NCORES)))
    outs = [res.results[i]["out"].reshape(BL, 1) for i in range(NCORES)]
    return np.concatenate(outs, axis=0).astype(np.float32)


# revision 23
# speedup vs baseline: 1.2888x; 1.0276x over previous
"""Trainium2 Bass kernel: ConvLSTM1D -> BiLSTM -> dense sigmoid.

Reference model (per full batch B=32):
  h = ConvLSTM1D(x (B,64,512,32); k (2,32,128) stride2, r (2,32,128), hard_sigmoid)
      -> final hidden (B, 256, 32)
  hf = LSTM(h) last state; hb = LSTM(h reversed) last state  (U=32 each)
  out = sigmoid(concat(hf,hb) @ w_d + b_d)   (B, 1)

Sharding: pure data parallelism, batch 32 -> 8 cores x 4.

Per-core layout choices:
  ConvLSTM scan state/gates: partitions = (b4, ch32) = 128, free = j (256).
    Matmuls use block-diagonal weights lhsT[(b',cin),(b,ch)] = delta_bb' W[cin,ch]
    (K=128, M=128, N=256, float32r -> 1 cycle/row) accumulating input-conv taps
    and recurrent-conv taps into one PSUM group per gate.
  BiLSTM: transposed layout, partitions = (gate,U) = 128, free = batch (4).
    Two interleaved chains (fwd, bwd); zx injected by identity-matmul.
Gate order is host-reordered from Keras (i,f,g,o) to (i,f,o,g) so the three
hard-sigmoid/sigmoid gates are contiguous.
"""

import numpy as np

import concourse.bass as bass
import concourse.bacc as bacc
import concourse.mybir as mybir
from concourse.tile import TileContext
from concourse.bass_utils import run_bass_kernel_spmd

B, T, L, C = 32, 64, 512, 32
F = 32          # conv filters
U = 32          # lstm units
NCORES = 8
BL = B // NCORES          # 4 local batch
LO = L // 2               # 256 spatial after stride-2 conv
G4 = 4 * F                # 128 gate channels

FP = mybir.dt.float32
BF = mybir.dt.bfloat16

# w_bf column layout (bf16): big matmul weights
#  [0:2048)    16 block-diag (128x128) conv weights, index (g*2+tap)*128,
#              first 8 = input conv, next 8 = recurrent conv
#  [2048:2176) identity 128x128
#  [2176:3200) 8 block-diag zx weights bdk[d][g][(b,ch),(b,U)]
#  [3200:4224) 8 block-diag lstm rec weights bdr[d][g][(b,U'),(b,U)]
#  [4224:4232) dense wdx[d] (128,4): [(b,u), b] = delta * w_d[u+32d]
WBF_COLS = 4232
# w_all column layout (f32): biases
#  [0:8)       lstm biases per (d,g): (128,1) = b_d[g*32+u]
#  [8]         0.5 constant
#  [9]         b_d (dense bias) replicated
W_COLS = 10

_CACHE = {}


def _reorder_gates(w, n):
    # last dim (4n): keras order i,f,g,o -> i,f,o,g
    i, f, g, o = np.split(w, 4, axis=-1)
    return np.concatenate([i, f, o, g], axis=-1)


def _build_graph():
    nc = bacc.Bacc("TRN2")
    x2 = nc.declare_dram_parameter("x2", [128, T, 2 * LO], BF, isOutput=False)
    w_bf = nc.declare_dram_parameter("w_bf", [128, WBF_COLS], BF, isOutput=False)
    w_all = nc.declare_dram_parameter("w_all", [128, W_COLS], FP, isOutput=False)
    out = nc.declare_dram_parameter("out", [BL, 1], FP, isOutput=True)

    AF = mybir.ActivationFunctionType
    ALU = mybir.AluOpType

    with TileContext(nc) as tc:
        with (
            tc.tile_pool(name="w", bufs=1) as wp,
            tc.tile_pool(name="x", bufs=4) as xp,
            tc.tile_pool(name="st", bufs=1) as sp,
            tc.tile_pool(name="g", bufs=3) as gp,
            tc.tile_pool(name="gb", bufs=8) as gpb,
            tc.tile_pool(name="zp", bufs=4, space="PSUM") as zp,
        ):
            W = wp.tile([128, W_COLS], FP)
            nc.sync.dma_start(out=W[:], in_=w_all[:])
            WB = wp.tile([128, WBF_COLS], BF)
            nc.sync.dma_start(out=WB[:], in_=w_bf[:])

            def wconv(idx):  # (128,128) bf16 block-diag conv weight
                return WB[:, idx * 128:(idx + 1) * 128]

            ident = WB[:, 2048:2176]

            def bdk(d, g):  # zx input weights, block-diag (bf16)
                o = 2176 + (d * 4 + g) * 128
                return WB[:, o:o + 128]

            def bdr(d, g):  # lstm recurrent weights, block-diag (bf16)
                o = 3200 + (d * 4 + g) * 128
                return WB[:, o:o + 128]

            wdx = [WB[:, 4224:4228], WB[:, 4228:4232]]
            bls = [[W[:, d * 4 + g:d * 4 + g + 1] for g in range(4)]
                   for d in range(2)]
            half = W[:, 8:9]
            bd = W[0:4, 9:10]

            # ---------------- Phase A: ConvLSTM scan over T ----------------
            h_sb = sp.tile([128, LO + 1], BF)   # col 256 stays zero (pad)
            c_sb = sp.tile([128, LO], FP)
            nc.vector.memset(h_sb[:, LO:LO + 1], 0.0)

            # two PSUM tiles (one bank each) so gate reads never falsely
            # serialize against later gates' matmul writes:
            # zA = [g | f], zB = [i | o]; emission order g, i, f, o
            for t in range(T):
                xt = xp.tile([128, 2, LO], BF, tag="xt")
                nc.sync.dma_start(out=xt[:], in_=x2[:, t, :])
                zA = zp.tile([128, 2 * LO], FP, tag="zA")
                zB = zp.tile([128, 2 * LO], FP, tag="zB")
                sig = gp.tile([128, 3, LO], BF, tag="sig")
                tg = gp.tile([128, LO], BF, tag="tg")
                tc_t = gp.tile([128, LO], BF, tag="tc")
                tmp = gp.tile([128, LO], BF, tag="tmp")
                c2 = gp.tile([128, LO], FP, tag="c2")

                def convmm(g, zt_, pos):
                    zg = zt_[:, pos * LO:(pos + 1) * LO]
                    n_acc = 4 if t > 0 else 2
                    k = 0
                    for tap in range(2):
                        nc.tensor.matmul(
                            zg, lhsT=wconv(g * 2 + tap), rhs=xt[:, tap, :],
                            start=(k == 0), stop=(k == n_acc - 1))
                        k += 1
                    if t > 0:
                        for tap in range(2):
                            nc.tensor.matmul(
                                zg, lhsT=wconv(8 + g * 2 + tap),
                                rhs=h_sb[:, tap:tap + LO],
                                start=False, stop=(k == n_acc - 1))
                            k += 1

                # gate index in weights: 0=i 1=f 2=o 3=g (host order i,f,o,g)
                convmm(3, zA, 0)
                nc.scalar.activation(tg[:], zA[:, 0:LO], AF.Tanh)
                convmm(0, zB, 0)
                nc.scalar.activation(sig[:, 0, :], zB[:, 0:LO],
                                     AF.Relu, bias=half, scale=0.2)
                # tmp = min(sig_i,1) * tanh(zg)
                nc.vector.scalar_tensor_tensor(
                    (c_sb[:] if t == 0 else tmp[:]),
                    sig[:, 0, :], 1.0, tg[:], ALU.min, ALU.mult)
                convmm(1, zA, 1)
                nc.scalar.activation(sig[:, 1, :], zA[:, LO:2 * LO],
                                     AF.Relu, bias=half, scale=0.2)
                if t > 0:
                    nc.vector.scalar_tensor_tensor(
                        c2[:], sig[:, 1, :], 1.0, c_sb[:], ALU.min, ALU.mult)
                    nc.vector.tensor_tensor(c_sb[:], tmp[:], c2[:], ALU.add)
                convmm(2, zB, 1)
                nc.scalar.activation(sig[:, 2, :], zB[:, LO:2 * LO],
                                     AF.Relu, bias=half, scale=0.2)
                nc.scalar.activation(tc_t[:], c_sb[:], AF.Tanh)
                nc.vector.scalar_tensor_tensor(
                    h_sb[:, 0:LO], sig[:, 2, :], 1.0, tc_t[:],
                    ALU.min, ALU.mult)

            # ---------------- Phase B: bidirectional LSTM over LO ----------
            # Layout: partitions = (b,U) = 128, free = gate cols. No partition
            # shifts anywhere (walrus verifier requires same partitions).
            # zx[d][g] (128, LO): input-side gates + lstm bias, injected into
            # the per-step PSUM via identity matmul (i,f,o) / ACT bias (g).
            zxs = []
            for d in range(2):
                psA = zp.tile([128, 2 * LO], FP, tag="zA", name=f"zxpsA{d}")
                psB = zp.tile([128, 2 * LO], FP, tag="zB", name=f"zxpsB{d}")
                pss = [psA, psB]

                def ps_slice(g):
                    return pss[g // 2][:, (g % 2) * LO:(g % 2 + 1) * LO]

                for g in range(4):
                    nc.tensor.matmul(
                        ps_slice(g), lhsT=bdk(d, g),
                        rhs=h_sb[:, 0:LO],
                        start=True, stop=True)
                zx_ifo = sp.tile([128, LO, 3], BF, tag=f"zxifo{d}",
                                 name=f"zxifo{d}")
                zx_g = sp.tile([128, LO], FP, tag=f"zxg{d}", name=f"zxg{d}")
                # evacuation + lstm-bias fold; split across ACT and DVE
                nc.scalar.activation(
                    zx_ifo[:, :, 0], ps_slice(0), AF.Identity, bias=bls[d][0])
                nc.vector.scalar_tensor_tensor(
                    zx_ifo[:, :, 1], ps_slice(1), bls[d][1],
                    h_sb[:, 0:LO], ALU.add, ALU.bypass)
                nc.scalar.activation(
                    zx_ifo[:, :, 2], ps_slice(2), AF.Identity,
                    bias=bls[d][2])
                nc.vector.scalar_tensor_tensor(
                    zx_g[:], ps_slice(3), bls[d][3],
                    h_sb[:, 0:LO], ALU.add, ALU.bypass)
                zxs.append((zx_ifo, zx_g))

            # state: hT[d] bf16 (feeds bf16 matmul), cT[d] f32
            hT = [sp.tile([128, 1], BF, tag=f"hT{d}", name=f"hT{d}")
                  for d in range(2)]
            cT = [sp.tile([128, 1], FP, tag=f"cT{d}", name=f"cT{d}")
                  for d in range(2)]

            def pb_mm(s, d):
                se = s if d == 0 else LO - 1 - s
                zx_ifo, _ = zxs[d]
                # fresh PSUM slots; zifo and zg in different banks (zA/zB)
                zifo = zp.tile([128, 2 * LO], FP, tag="zA",
                               name=f"zi{d}")[:, 0:3]
                zg = zp.tile([128, 2 * LO], FP, tag="zB",
                             name=f"zgt{d}")[:, 0:1]
                # inject first: it has no dependency on h, runs ahead
                nc.tensor.matmul(zifo, lhsT=ident,
                                 rhs=zx_ifo[:, se, :],
                                 start=True, stop=(s == 0),
                                 skip_group_check=True)
                if s > 0:
                    nc.tensor.matmul(zg, lhsT=bdr(d, 3), rhs=hT[d][:],
                                     start=True, stop=True,
                                     skip_group_check=True)
                    for g in range(3):
                        nc.tensor.matmul(
                            zifo[:, g:g + 1], lhsT=bdr(d, g),
                            rhs=hT[d][:], start=False, stop=(g == 2),
                            skip_group_check=True)
                return zifo, zg, se

            for s in range(LO):
                zz = [pb_mm(s, 0), pb_mm(s, 1)]
                tl = []
                for d in range(2):
                    tl.append((gpb.tile([128, 3], BF, tag=f"sg{d}",
                                        name=f"sg{d}"),
                               gpb.tile([128, 1], BF, tag=f"tg{d}",
                                        name=f"tg{d}"),
                               gpb.tile([128, 1], BF, tag=f"tc{d}",
                                        name=f"tc{d}"),
                               gpb.tile([128, 1], FP, tag=f"tm1{d}",
                                        name=f"tm1{d}")))
                # interleave the two chains op-by-op on each engine
                for d in range(2):
                    zifo, zg, se = zz[d]
                    sg, tgl, tcl, tm1 = tl[d]
                    zx_g = zxs[d][1]
                    if s > 0:
                        nc.scalar.activation(tgl[:], zg, AF.Tanh,
                                             bias=zx_g[:, se:se + 1])
                    else:
                        nc.scalar.activation(tgl[:], zx_g[:, se:se + 1],
                                             AF.Tanh)
                    nc.scalar.activation(sg[:], zifo, AF.Sigmoid)
                    # tm1 = sig_i * tanh_g
                    nc.vector.scalar_tensor_tensor(
                        tm1[:], sg[:, 0:1], tgl[:], sg[:, 0:1],
                        ALU.mult, ALU.bypass)
                    if s > 0:
                        nc.vector.scalar_tensor_tensor(
                            cT[d][:], sg[:, 1:2], cT[d][:], tm1[:],
                            ALU.mult, ALU.add)
                    else:
                        nc.vector.tensor_copy(cT[d][:], tm1[:])
                for d in range(2):
                    sg, tgl, tcl, tm1 = tl[d]
                    nc.scalar.activation(tcl[:], cT[d][:], AF.Tanh)
                    nc.vector.scalar_tensor_tensor(
                        hT[d][:], sg[:, 2:3], tcl[:], sg[:, 2:3],
                        ALU.mult, ALU.bypass)

            # ---------------- dense + sigmoid ----------------
            fo = zp.tile([128, 2 * LO], FP, tag="zA",
                         name="fo")[0:BL, 0:1]
            nc.tensor.matmul(fo, lhsT=wdx[0], rhs=hT[0][:],
                             start=True, stop=False, skip_group_check=True)
            nc.tensor.matmul(fo, lhsT=wdx[1], rhs=hT[1][:],
                             start=False, stop=True, skip_group_check=True)
            res = gp.tile([BL, 1], FP, tag="res")
            nc.scalar.activation(res[:], fo, AF.Sigmoid, bias=bd)
            nc.sync.dma_start(out=out[:], in_=res[:])

    nc.compile()
    return nc


def _prep_inputs(x, k_conv, r_conv, b_conv, k_f, r_f, b_f, k_b, r_b, b_b,
                 w_d, b_d):
    """Host-side: gate reorder, block-diag expansion, x transpose."""
    assert np.all(b_conv == 0.0), "nonzero b_conv not supported by this kernel"
    k_conv = _reorder_gates(np.asarray(k_conv, np.float32), F)
    r_conv = _reorder_gates(np.asarray(r_conv, np.float32), F)
    k_f = _reorder_gates(np.asarray(k_f, np.float32), U)
    r_f = _reorder_gates(np.asarray(r_f, np.float32), U)
    b_f = _reorder_gates(np.asarray(b_f, np.float32), U)
    k_b = _reorder_gates(np.asarray(k_b, np.float32), U)
    r_b = _reorder_gates(np.asarray(r_b, np.float32), U)
    b_b = _reorder_gates(np.asarray(b_b, np.float32), U)

    import ml_dtypes
    w_bf = np.zeros((128, WBF_COLS), np.float32)
    w_all = np.zeros((128, W_COLS), np.float32)
    for g in range(4):
        for tap in range(2):
            wi = np.zeros((128, 128), np.float32)
            wr = np.zeros((128, 128), np.float32)
            for b in range(4):
                sl = slice(b * 32, (b + 1) * 32)
                wi[sl, sl] = k_conv[tap, :, g * 32:(g + 1) * 32]
                wr[sl, sl] = r_conv[tap, :, g * 32:(g + 1) * 32]
            w_bf[:, (g * 2 + tap) * 128:(g * 2 + tap + 1) * 128] = wi
            w_bf[:, (8 + g * 2 + tap) * 128:(9 + g * 2 + tap) * 128] = wr
    w_bf[:, 2048:2176] = np.eye(128, dtype=np.float32)
    w_d = np.asarray(w_d, np.float32)
    for d, (kk, rr, bb) in enumerate([(k_f, r_f, b_f), (k_b, r_b, b_b)]):
        for g in range(4):
            bk = np.zeros((128, 128), np.float32)
            br = np.zeros((128, 128), np.float32)
            for b in range(4):
                sl = slice(b * 32, (b + 1) * 32)
                bk[sl, sl] = kk[:, g * 32:(g + 1) * 32]
                br[sl, sl] = rr[:, g * 32:(g + 1) * 32]
            w_bf[:, 2176 + (d * 4 + g) * 128:2304 + (d * 4 + g) * 128] = bk
            w_bf[:, 3200 + (d * 4 + g) * 128:3328 + (d * 4 + g) * 128] = br
            w_all[:, d * 4 + g] = np.tile(bb[g * 32:(g + 1) * 32], 4)
        wx = np.zeros((128, 4), np.float32)
        for b in range(4):
            wx[b * 32:(b + 1) * 32, b] = w_d[d * 32:(d + 1) * 32, 0]
        w_bf[:, 4224 + d * 4:4228 + d * 4] = wx
    w_all[:, 8] = 0.5
    w_all[0:4, 9] = np.float32(np.asarray(b_d).reshape(-1)[0])
    w_bf = w_bf.astype(ml_dtypes.bfloat16)

    # x (B,T,512,C) -> per-core (128=(b,c), T, (tap,j)): x2[b*32+c, t, tap*256+j]
    #   = x[b, t, 2j+tap, c]
    x = np.asarray(x, np.float32).reshape(B, T, LO, 2, C)
    # -> (B, C, T, tap, j)
    xt = np.ascontiguousarray(x.transpose(0, 4, 1, 3, 2))
    x2_full = xt.reshape(B * C, T, 2 * LO)

    x2_full = x2_full.astype(ml_dtypes.bfloat16)
    in_maps = []
    for core in range(NCORES):
        x2c = np.ascontiguousarray(
            x2_full[core * BL * C:(core + 1) * BL * C])
        in_maps.append({"x2": x2c, "w_bf": w_bf, "w_all": w_all})
    return in_maps


def kernel(**inputs) -> np.ndarray:
    if "nc" not in _CACHE:
        _CACHE["nc"] = _build_graph()
    nc = _CACHE["nc"]
    in_maps = _prep_inputs(**inputs)
    res = run_bass_kernel_spmd(nc, in_maps, core_ids=list(range(NCORES)))
    outs = [res.results[i]["out"].reshape(BL, 1) for i in range(NCORES)]
    return np.concatenate(outs, axis=0).astype(np.float32)


# revision 26
# speedup vs baseline: 1.3320x; 1.0335x over previous
"""Trainium2 Bass kernel: ConvLSTM1D -> BiLSTM -> dense sigmoid.

Reference model (per full batch B=32):
  h = ConvLSTM1D(x (B,64,512,32); k (2,32,128) stride2, r (2,32,128), hard_sigmoid)
      -> final hidden (B, 256, 32)
  hf = LSTM(h) last state; hb = LSTM(h reversed) last state  (U=32 each)
  out = sigmoid(concat(hf,hb) @ w_d + b_d)   (B, 1)

Sharding: pure data parallelism, batch 32 -> 8 cores x 4.

Per-core layout choices:
  ConvLSTM scan state/gates: partitions = (b4, ch32) = 128, free = j (256).
    Matmuls use block-diagonal weights lhsT[(b',cin),(b,ch)] = delta_bb' W[cin,ch]
    (K=128, M=128, N=256, float32r -> 1 cycle/row) accumulating input-conv taps
    and recurrent-conv taps into one PSUM group per gate.
  BiLSTM: transposed layout, partitions = (gate,U) = 128, free = batch (4).
    Two interleaved chains (fwd, bwd); zx injected by identity-matmul.
Gate order is host-reordered from Keras (i,f,g,o) to (i,f,o,g) so the three
hard-sigmoid/sigmoid gates are contiguous.
"""

import numpy as np

import concourse.bass as bass
import concourse.bacc as bacc
import concourse.mybir as mybir
from concourse.tile import TileContext
from concourse.bass_utils import run_bass_kernel_spmd

B, T, L, C = 32, 64, 512, 32
F = 32          # conv filters
U = 32          # lstm units
NCORES = 8
BL = B // NCORES          # 4 local batch
LO = L // 2               # 256 spatial after stride-2 conv
G4 = 4 * F                # 128 gate channels

FP = mybir.dt.float32
BF = mybir.dt.bfloat16

# w_bf column layout (bf16): big matmul weights
#  [0:2048)    16 block-diag (128x128) conv weights, index (g*2+tap)*128,
#              first 8 = input conv, next 8 = recurrent conv
#  [2048:2176) identity 128x128
#  [2176:3200) 8 block-diag zx weights bdk[d][g][(b,ch),(b,U)]
#  [3200:4224) 8 block-diag lstm rec weights bdr[d][g][(b,U'),(b,U)]
#  [4224:4232) dense wdx[d] (128,4): [(b,u), b] = delta * w_d[u+32d]
WBF_COLS = 4232
# w_all column layout (f32): biases
#  [0:8)       lstm biases per (d,g): (128,1) = b_d[g*32+u]
#  [8]         0.5 constant
#  [9]         b_d (dense bias) replicated
W_COLS = 10

_CACHE = {}


def _reorder_gates(w, n):
    # last dim (4n): keras order i,f,g,o -> i,f,o,g
    i, f, g, o = np.split(w, 4, axis=-1)
    return np.concatenate([i, f, o, g], axis=-1)


def _build_graph():
    nc = bacc.Bacc("TRN2")
    x2 = nc.declare_dram_parameter("x2", [128, T, 2 * LO], BF, isOutput=False)
    w_bf = nc.declare_dram_parameter("w_bf", [128, WBF_COLS], BF, isOutput=False)
    w_all = nc.declare_dram_parameter("w_all", [128, W_COLS], FP, isOutput=False)
    out = nc.declare_dram_parameter("out", [BL, 1], FP, isOutput=True)

    AF = mybir.ActivationFunctionType
    ALU = mybir.AluOpType

    with TileContext(nc) as tc:
        with (
            tc.tile_pool(name="w", bufs=1) as wp,
            tc.tile_pool(name="x", bufs=4) as xp,
            tc.tile_pool(name="st", bufs=1) as sp,
            tc.tile_pool(name="g", bufs=3) as gp,
            tc.tile_pool(name="gb", bufs=8) as gpb,
            tc.tile_pool(name="zp", bufs=2, space="PSUM") as zp,
        ):
            W = wp.tile([128, W_COLS], FP)
            nc.sync.dma_start(out=W[:], in_=w_all[:])
            WB = wp.tile([128, WBF_COLS], BF)
            nc.sync.dma_start(out=WB[:], in_=w_bf[:])

            def wconv(idx):  # (128,128) bf16 block-diag conv weight
                return WB[:, idx * 128:(idx + 1) * 128]

            ident = WB[:, 2048:2176]

            def bdk(d, g):  # zx input weights, block-diag (bf16)
                o = 2176 + (d * 4 + g) * 128
                return WB[:, o:o + 128]

            def bdr(d, g):  # lstm recurrent weights, block-diag (bf16)
                o = 3200 + (d * 4 + g) * 128
                return WB[:, o:o + 128]

            wdx = [WB[:, 4224:4228], WB[:, 4228:4232]]
            bls = [[W[:, d * 4 + g:d * 4 + g + 1] for g in range(4)]
                   for d in range(2)]
            half = W[:, 8:9]
            bd = W[0:4, 9:10]

            # ---------------- Phase A: ConvLSTM scan over T ----------------
            h_sb = sp.tile([128, LO + 1], BF)   # col 256 stays zero (pad)
            c_sb = sp.tile([128, LO], FP)
            nc.vector.memset(h_sb[:, LO:LO + 1], 0.0)

            # two PSUM tiles (one bank each) so gate reads never falsely
            # serialize against later gates' matmul writes:
            # zA = [g | f], zB = [i | o]; emission order g, i, f, o
            for t in range(T):
                xt = xp.tile([128, 2, LO], BF, tag="xt")
                nc.sync.dma_start(out=xt[:], in_=x2[:, t, :])
                zt4 = [zp.tile([128, LO], FP, tag=f"az{g}",
                               name=f"az{g}") for g in range(4)]
                sig = gp.tile([128, 3, LO], BF, tag="sig")
                tg = gp.tile([128, LO], BF, tag="tg")
                tc_t = gp.tile([128, LO], BF, tag="tc")
                tmp = gp.tile([128, LO], BF, tag="tmp")
                c2 = gp.tile([128, LO], FP, tag="c2")

                def conv_inp(g, zg):
                    for tap in range(2):
                        nc.tensor.matmul(
                            zg[:], lhsT=wconv(g * 2 + tap), rhs=xt[:, tap, :],
                            start=(tap == 0),
                            stop=(t == 0 and tap == 1))

                def conv_rec(g, zg):
                    for tap in range(2):
                        nc.tensor.matmul(
                            zg[:], lhsT=wconv(8 + g * 2 + tap),
                            rhs=h_sb[:, tap:tap + LO],
                            start=False, stop=(tap == 1))

                # gate index in weights: 0=i 1=f 2=o 3=g (host order i,f,o,g)
                # psum tile index: zt4[0]=g zt4[1]=i zt4[2]=f zt4[3]=o
                # all input-side matmuls first: they have no h dependency, so
                # the in-order PE queue fills the previous step's gate tail
                for g_, p_ in ((3, 0), (0, 1), (1, 2), (2, 3)):
                    conv_inp(g_, zt4[p_])
                if t > 0:
                    conv_rec(3, zt4[0])
                nc.scalar.activation(tg[:], zt4[0][:], AF.Tanh)
                if t > 0:
                    conv_rec(0, zt4[1])
                nc.scalar.activation(sig[:, 0, :], zt4[1][:],
                                     AF.Relu, bias=half, scale=0.2)
                # tmp = min(sig_i,1) * tanh(zg)
                nc.vector.scalar_tensor_tensor(
                    (c_sb[:] if t == 0 else tmp[:]),
                    sig[:, 0, :], 1.0, tg[:], ALU.min, ALU.mult)
                if t > 0:
                    conv_rec(1, zt4[2])
                nc.scalar.activation(sig[:, 1, :], zt4[2][:],
                                     AF.Relu, bias=half, scale=0.2)
                if t > 0:
                    nc.vector.scalar_tensor_tensor(
                        c2[:], sig[:, 1, :], 1.0, c_sb[:], ALU.min, ALU.mult)
                    nc.vector.tensor_tensor(c_sb[:], tmp[:], c2[:], ALU.add)
                if t > 0:
                    conv_rec(2, zt4[3])
                nc.scalar.activation(sig[:, 2, :], zt4[3][:],
                                     AF.Relu, bias=half, scale=0.2)
                nc.scalar.activation(tc_t[:], c_sb[:], AF.Tanh)
                nc.vector.scalar_tensor_tensor(
                    h_sb[:, 0:LO], sig[:, 2, :], 1.0, tc_t[:],
                    ALU.min, ALU.mult)

            # ---------------- Phase B: bidirectional LSTM over LO ----------
            # Layout: partitions = (b,U) = 128, free = gate cols. No partition
            # shifts anywhere (walrus verifier requires same partitions).
            # zx[d][g] (128, LO): input-side gates + lstm bias, injected into
            # the per-step PSUM via identity matmul (i,f,o) / ACT bias (g).
            zxs = []
            for d in range(2):
                pss = [zp.tile([128, LO], FP, tag=f"az{g}",
                               name=f"zxps{g}") for g in range(4)]

                def ps_slice(g):
                    return pss[g][:]

                for g in range(4):
                    nc.tensor.matmul(
                        ps_slice(g), lhsT=bdk(d, g),
                        rhs=h_sb[:, 0:LO],
                        start=True, stop=True)
                zx4 = sp.tile([128, LO, 4], BF, tag=f"zx4{d}",
                              name=f"zx4{d}")
                # evacuation + lstm-bias fold; split across ACT and DVE.
                # gate col 3 (g) is pre-scaled x2 on the host: tanh(x) is
                # computed as 2*sigmoid(2x)-1 so one sigmoid covers all gates.
                nc.scalar.activation(
                    zx4[:, :, 0], ps_slice(0), AF.Identity, bias=bls[d][0])
                nc.vector.scalar_tensor_tensor(
                    zx4[:, :, 1], ps_slice(1), bls[d][1],
                    h_sb[:, 0:LO], ALU.add, ALU.bypass)
                nc.scalar.activation(
                    zx4[:, :, 2], ps_slice(2), AF.Identity,
                    bias=bls[d][2])
                nc.vector.scalar_tensor_tensor(
                    zx4[:, :, 3], ps_slice(3), bls[d][3],
                    h_sb[:, 0:LO], ALU.add, ALU.bypass)
                zxs.append(zx4)

            # state: hT[d] bf16 (feeds bf16 matmul), cT[d] f32
            hT = [sp.tile([128, 1], BF, tag=f"hT{d}", name=f"hT{d}")
                  for d in range(2)]
            cT = [sp.tile([128, 1], FP, tag=f"cT{d}", name=f"cT{d}")
                  for d in range(2)]

            def pb_mm(s, d):
                se = s if d == 0 else LO - 1 - s
                zx4 = zxs[d]
                # fresh PSUM slot per (s, d): no cross-step bank conflicts
                z4 = zp.tile([128, LO], FP, tag=f"az{d}",
                             name=f"z4{d}")[:, 0:4]
                # inject first: it has no dependency on h, runs ahead
                nc.tensor.matmul(z4, lhsT=ident,
                                 rhs=zx4[:, se, :],
                                 start=True, stop=(s == 0),
                                 skip_group_check=True)
                if s > 0:
                    for k, g in enumerate((3, 0, 1, 2)):
                        nc.tensor.matmul(
                            z4[:, g:g + 1], lhsT=bdr(d, g),
                            rhs=hT[d][:], start=False, stop=(k == 3),
                            skip_group_check=True)
                return z4, se

            for s in range(LO):
                zz = [pb_mm(s, 0), pb_mm(s, 1)]
                # gate cols: 0=i 1=f 2=o 3=g' (sigmoid of 2x)
                tl = []
                for d in range(2):
                    tl.append((gpb.tile([128, 4], BF, tag=f"sg{d}",
                                        name=f"sg{d}"),
                               gpb.tile([128, 1], BF, tag=f"tg{d}",
                                        name=f"tg{d}"),
                               gpb.tile([128, 1], BF, tag=f"tc{d}",
                                        name=f"tc{d}"),
                               gpb.tile([128, 1], FP, tag=f"tm1{d}",
                                        name=f"tm1{d}")))
                # interleave the two chains op-by-op on each engine
                for d in range(2):
                    z4, se = zz[d]
                    sg, tgl, tcl, tm1 = tl[d]
                    nc.scalar.activation(sg[:], z4, AF.Sigmoid)
                    # tanh(zg) = 2*sigmoid(2 zg) - 1
                    nc.vector.tensor_scalar(
                        tgl[:], sg[:, 3:4], 2.0, 1.0,
                        ALU.mult, ALU.subtract)
                    # tm1 = sig_i * tanh_g
                    nc.vector.scalar_tensor_tensor(
                        tm1[:], sg[:, 0:1], tgl[:], sg[:, 0:1],
                        ALU.mult, ALU.bypass)
                    if s > 0:
                        nc.vector.scalar_tensor_tensor(
                            cT[d][:], sg[:, 1:2], cT[d][:], tm1[:],
                            ALU.mult, ALU.add)
                    else:
                        nc.vector.tensor_copy(cT[d][:], tm1[:])
                for d in range(2):
                    sg, tgl, tcl, tm1 = tl[d]
                    nc.scalar.activation(tcl[:], cT[d][:], AF.Tanh)
                    nc.vector.scalar_tensor_tensor(
                        hT[d][:], sg[:, 2:3], tcl[:], sg[:, 2:3],
                        ALU.mult, ALU.bypass)

            # ---------------- dense + sigmoid ----------------
            fo = zp.tile([128, LO], FP, tag="az2",
                         name="fo")[0:BL, 0:1]
            nc.tensor.matmul(fo, lhsT=wdx[0], rhs=hT[0][:],
                             start=True, stop=False, skip_group_check=True)
            nc.tensor.matmul(fo, lhsT=wdx[1], rhs=hT[1][:],
                             start=False, stop=True, skip_group_check=True)
            res = gp.tile([BL, 1], FP, tag="res")
            nc.scalar.activation(res[:], fo, AF.Sigmoid, bias=bd)
            nc.sync.dma_start(out=out[:], in_=res[:])

    nc.compile()
    return nc


def _prep_inputs(x, k_conv, r_conv, b_conv, k_f, r_f, b_f, k_b, r_b, b_b,
                 w_d, b_d):
    """Host-side: gate reorder, block-diag expansion, x transpose."""
    assert np.all(b_conv == 0.0), "nonzero b_conv not supported by this kernel"
    k_conv = _reorder_gates(np.asarray(k_conv, np.float32), F)
    r_conv = _reorder_gates(np.asarray(r_conv, np.float32), F)
    k_f = _reorder_gates(np.asarray(k_f, np.float32), U)
    r_f = _reorder_gates(np.asarray(r_f, np.float32), U)
    b_f = _reorder_gates(np.asarray(b_f, np.float32), U)
    k_b = _reorder_gates(np.asarray(k_b, np.float32), U)
    r_b = _reorder_gates(np.asarray(r_b, np.float32), U)
    b_b = _reorder_gates(np.asarray(b_b, np.float32), U)

    import ml_dtypes
    w_bf = np.zeros((128, WBF_COLS), np.float32)
    w_all = np.zeros((128, W_COLS), np.float32)
    for g in range(4):
        for tap in range(2):
            wi = np.zeros((128, 128), np.float32)
            wr = np.zeros((128, 128), np.float32)
            for b in range(4):
                sl = slice(b * 32, (b + 1) * 32)
                wi[sl, sl] = k_conv[tap, :, g * 32:(g + 1) * 32]
                wr[sl, sl] = r_conv[tap, :, g * 32:(g + 1) * 32]
            w_bf[:, (g * 2 + tap) * 128:(g * 2 + tap + 1) * 128] = wi
            w_bf[:, (8 + g * 2 + tap) * 128:(9 + g * 2 + tap) * 128] = wr
    w_bf[:, 2048:2176] = np.eye(128, dtype=np.float32)
    w_d = np.asarray(w_d, np.float32)
    for d, (kk, rr, bb) in enumerate([(k_f, r_f, b_f), (k_b, r_b, b_b)]):
        for g in range(4):
            bk = np.zeros((128, 128), np.float32)
            br = np.zeros((128, 128), np.float32)
            for b in range(4):
                sl = slice(b * 32, (b + 1) * 32)
                bk[sl, sl] = kk[:, g * 32:(g + 1) * 32]
                br[sl, sl] = rr[:, g * 32:(g + 1) * 32]
            pre = 2.0 if g == 3 else 1.0   # tanh-as-sigmoid prescale
            w_bf[:, 2176 + (d * 4 + g) * 128:2304 + (d * 4 + g) * 128] = \
                bk * pre
            w_bf[:, 3200 + (d * 4 + g) * 128:3328 + (d * 4 + g) * 128] = \
                br * pre
            w_all[:, d * 4 + g] = np.tile(bb[g * 32:(g + 1) * 32], 4) * pre
        wx = np.zeros((128, 4), np.float32)
        for b in range(4):
            wx[b * 32:(b + 1) * 32, b] = w_d[d * 32:(d + 1) * 32, 0]
        w_bf[:, 4224 + d * 4:4228 + d * 4] = wx
    w_all[:, 8] = 0.5
    w_all[0:4, 9] = np.float32(np.asarray(b_d).reshape(-1)[0])
    w_bf = w_bf.astype(ml_dtypes.bfloat16)

    # x (B,T,512,C) -> per-core (128=(b,c), T, (tap,j)): x2[b*32+c, t, tap*256+j]
    #   = x[b, t, 2j+tap, c]
    x = np.asarray(x, np.float32).reshape(B, T, LO, 2, C)
    # -> (B, C, T, tap, j)
    xt = np.ascontiguousarray(x.transpose(0, 4, 1, 3, 2))
    x2_full = xt.reshape(B * C, T, 2 * LO)

    x2_full = x2_full.astype(ml_dtypes.bfloat16)
    in_maps = []
    for core in range(NCORES):
        x2c = np.ascontiguousarray(
            x2_full[core * BL * C:(core + 1) * BL * C])
        in_maps.append({"x2": x2c, "w_bf": w_bf, "w_all": w_all})
    return in_maps


def kernel(**inputs) -> np.ndarray:
    if "nc" not in _CACHE:
        _CACHE["nc"] = _build_graph()
    nc = _CACHE["nc"]
    in_maps = _prep_inputs(**inputs)
    res = run_bass_kernel_spmd(nc, in_maps, core_ids=list(range(NCORES)))
    outs = [res.results[i]["out"].reshape(BL, 1) for i in range(NCORES)]
    return np.concatenate(outs, axis=0).astype(np.float32)


# revision 27
# speedup vs baseline: 1.3533x; 1.0160x over previous
"""Trainium2 Bass kernel: ConvLSTM1D -> BiLSTM -> dense sigmoid.

Reference model (per full batch B=32):
  h = ConvLSTM1D(x (B,64,512,32); k (2,32,128) stride2, r (2,32,128), hard_sigmoid)
      -> final hidden (B, 256, 32)
  hf = LSTM(h) last state; hb = LSTM(h reversed) last state  (U=32 each)
  out = sigmoid(concat(hf,hb) @ w_d + b_d)   (B, 1)

Sharding: pure data parallelism, batch 32 -> 8 cores x 4.

Per-core layout choices:
  ConvLSTM scan state/gates: partitions = (b4, ch32) = 128, free = j (256).
    Matmuls use block-diagonal weights lhsT[(b',cin),(b,ch)] = delta_bb' W[cin,ch]
    (K=128, M=128, N=256, float32r -> 1 cycle/row) accumulating input-conv taps
    and recurrent-conv taps into one PSUM group per gate.
  BiLSTM: transposed layout, partitions = (gate,U) = 128, free = batch (4).
    Two interleaved chains (fwd, bwd); zx injected by identity-matmul.
Gate order is host-reordered from Keras (i,f,g,o) to (i,f,o,g) so the three
hard-sigmoid/sigmoid gates are contiguous.
"""

import numpy as np

import concourse.bass as bass
import concourse.bacc as bacc
import concourse.mybir as mybir
from concourse.tile import TileContext
from concourse.bass_utils import run_bass_kernel_spmd

B, T, L, C = 32, 64, 512, 32
F = 32          # conv filters
U = 32          # lstm units
NCORES = 8
BL = B // NCORES          # 4 local batch
LO = L // 2               # 256 spatial after stride-2 conv
G4 = 4 * F                # 128 gate channels

FP = mybir.dt.float32
BF = mybir.dt.bfloat16

# w_bf column layout (bf16): big matmul weights
#  [0:2048)    16 block-diag (128x128) conv weights, index (g*2+tap)*128,
#              first 8 = input conv, next 8 = recurrent conv
#  [2048:2176) identity 128x128
#  [2176:3200) 8 block-diag zx weights bdk[d][g][(b,ch),(b,U)]
#  [3200:4224) 8 block-diag lstm rec weights bdr[d][g][(b,U'),(b,U)]
#  [4224:4232) dense wdx[d] (128,4): [(b,u), b] = delta * w_d[u+32d]
WBF_COLS = 4232
# w_all column layout (f32): biases
#  [0:8)       lstm biases per (d,g): (128,1) = b_d[g*32+u]
#  [8]         0.5 constant
#  [9]         b_d (dense bias) replicated
W_COLS = 10

_CACHE = {}


def _reorder_gates(w, n):
    # last dim (4n): keras order i,f,g,o -> i,f,o,g
    i, f, g, o = np.split(w, 4, axis=-1)
    return np.concatenate([i, f, o, g], axis=-1)


def _build_graph():
    nc = bacc.Bacc("TRN2")
    x2 = nc.declare_dram_parameter("x2", [128, T, 2 * LO], BF, isOutput=False)
    w_bf = nc.declare_dram_parameter("w_bf", [128, WBF_COLS], BF, isOutput=False)
    w_all = nc.declare_dram_parameter("w_all", [128, W_COLS], FP, isOutput=False)
    out = nc.declare_dram_parameter("out", [BL, 1], FP, isOutput=True)

    AF = mybir.ActivationFunctionType
    ALU = mybir.AluOpType

    with TileContext(nc) as tc:
        with (
            tc.tile_pool(name="w", bufs=1) as wp,
            tc.tile_pool(name="x", bufs=4) as xp,
            tc.tile_pool(name="st", bufs=1) as sp,
            tc.tile_pool(name="g", bufs=3) as gp,
            tc.tile_pool(name="gb", bufs=8) as gpb,
            tc.tile_pool(name="zp", bufs=2, space="PSUM") as zp,
        ):
            W = wp.tile([128, W_COLS], FP)
            nc.sync.dma_start(out=W[:], in_=w_all[:])
            WB = wp.tile([128, WBF_COLS], BF)
            nc.sync.dma_start(out=WB[:], in_=w_bf[:])

            def wconv(idx):  # (128,128) bf16 block-diag conv weight
                return WB[:, idx * 128:(idx + 1) * 128]

            ident = WB[:, 2048:2176]

            def bdk(d, g):  # zx input weights, block-diag (bf16)
                o = 2176 + (d * 4 + g) * 128
                return WB[:, o:o + 128]

            def bdr(d, g):  # lstm recurrent weights, block-diag (bf16)
                o = 3200 + (d * 4 + g) * 128
                return WB[:, o:o + 128]

            wdx = [WB[:, 4224:4228], WB[:, 4228:4232]]
            bls = [[W[:, d * 4 + g:d * 4 + g + 1] for g in range(4)]
                   for d in range(2)]
            half = W[:, 8:9]
            bd = W[0:4, 9:10]

            # ---------------- Phase A: ConvLSTM scan over T ----------------
            h_sb = sp.tile([128, LO + 1], BF)   # col 256 stays zero (pad)
            c_sb = sp.tile([128, LO], FP)
            nc.vector.memset(h_sb[:, LO:LO + 1], 0.0)

            # two PSUM tiles (one bank each) so gate reads never falsely
            # serialize against later gates' matmul writes:
            # zA = [g | f], zB = [i | o]; emission order g, i, f, o
            for t in range(T):
                xt = xp.tile([128, 2, LO], BF, tag="xt")
                nc.sync.dma_start(out=xt[:], in_=x2[:, t, :])
                zt4 = [zp.tile([128, LO], FP, tag=f"az{g}",
                               name=f"az{g}") for g in range(4)]
                sig = gp.tile([128, 3, LO], BF, tag="sig")
                tg = gp.tile([128, LO], BF, tag="tg")
                tc_t = gp.tile([128, LO], BF, tag="tc")
                tmp = gp.tile([128, LO], BF, tag="tmp")
                c2 = gp.tile([128, LO], FP, tag="c2")

                def conv_inp(g, zg):
                    for tap in range(2):
                        nc.tensor.matmul(
                            zg[:], lhsT=wconv(g * 2 + tap), rhs=xt[:, tap, :],
                            start=(tap == 0),
                            stop=(t == 0 and tap == 1))

                def conv_rec(g, zg):
                    for tap in range(2):
                        nc.tensor.matmul(
                            zg[:], lhsT=wconv(8 + g * 2 + tap),
                            rhs=h_sb[:, tap:tap + LO],
                            start=False, stop=(tap == 1))

                # gate index in weights: 0=i 1=f 2=o 3=g (host order i,f,o,g)
                # psum tile index: zt4[0]=g zt4[1]=i zt4[2]=f zt4[3]=o
                # all input-side matmuls first: they have no h dependency, so
                # the in-order PE queue fills the previous step's gate tail
                for g_, p_ in ((3, 0), (0, 1), (1, 2), (2, 3)):
                    conv_inp(g_, zt4[p_])
                if t > 0:
                    conv_rec(3, zt4[0])
                nc.scalar.activation(tg[:], zt4[0][:], AF.Tanh)
                if t > 0:
                    conv_rec(0, zt4[1])
                nc.scalar.activation(sig[:, 0, :], zt4[1][:],
                                     AF.Relu, bias=half, scale=0.2)
                # tmp = min(sig_i,1) * tanh(zg)
                nc.vector.scalar_tensor_tensor(
                    (c_sb[:] if t == 0 else tmp[:]),
                    sig[:, 0, :], 1.0, tg[:], ALU.min, ALU.mult)
                if t > 0:
                    conv_rec(1, zt4[2])
                nc.scalar.activation(sig[:, 1, :], zt4[2][:],
                                     AF.Relu, bias=half, scale=0.2)
                if t > 0:
                    nc.vector.scalar_tensor_tensor(
                        c2[:], sig[:, 1, :], 1.0, c_sb[:], ALU.min, ALU.mult)
                    nc.vector.tensor_tensor(c_sb[:], tmp[:], c2[:], ALU.add)
                if t > 0:
                    conv_rec(2, zt4[3])
                nc.scalar.activation(sig[:, 2, :], zt4[3][:],
                                     AF.Relu, bias=half, scale=0.2)
                nc.scalar.activation(tc_t[:], c_sb[:], AF.Tanh)
                nc.vector.scalar_tensor_tensor(
                    h_sb[:, 0:LO], sig[:, 2, :], 1.0, tc_t[:],
                    ALU.min, ALU.mult)

            # ---------------- Phase B: bidirectional LSTM over LO ----------
            # Layout: partitions = (b,U) = 128, free = gate cols. No partition
            # shifts anywhere (walrus verifier requires same partitions).
            # zx[d][g] (128, LO): input-side gates + lstm bias, injected into
            # the per-step PSUM via identity matmul (i,f,o) / ACT bias (g).
            zxs = []
            for d in range(2):
                pss = [zp.tile([128, LO], FP, tag=f"az{g}",
                               name=f"zxps{g}") for g in range(4)]

                def ps_slice(g):
                    return pss[g][:]

                for g in range(4):
                    nc.tensor.matmul(
                        ps_slice(g), lhsT=bdk(d, g),
                        rhs=h_sb[:, 0:LO],
                        start=True, stop=True)
                zx_ifo = sp.tile([128, LO, 3], BF, tag=f"zxifo{d}",
                                 name=f"zxifo{d}")
                zx_g = sp.tile([128, LO], FP, tag=f"zxg{d}", name=f"zxg{d}")
                # evacuation + lstm-bias fold; split across ACT and DVE
                nc.scalar.activation(
                    zx_ifo[:, :, 0], ps_slice(0), AF.Identity, bias=bls[d][0])
                nc.vector.scalar_tensor_tensor(
                    zx_ifo[:, :, 1], ps_slice(1), bls[d][1],
                    h_sb[:, 0:LO], ALU.add, ALU.bypass)
                nc.scalar.activation(
                    zx_ifo[:, :, 2], ps_slice(2), AF.Identity,
                    bias=bls[d][2])
                nc.vector.scalar_tensor_tensor(
                    zx_g[:], ps_slice(3), bls[d][3],
                    h_sb[:, 0:LO], ALU.add, ALU.bypass)
                zxs.append((zx_ifo, zx_g))

            # state: hT[d] bf16 (feeds bf16 matmul), cT[d] f32
            hT = [sp.tile([128, 1], BF, tag=f"hT{d}", name=f"hT{d}")
                  for d in range(2)]
            cT = [sp.tile([128, 1], FP, tag=f"cT{d}", name=f"cT{d}")
                  for d in range(2)]

            def pb_mm(s, d):
                se = s if d == 0 else LO - 1 - s
                zx_ifo, _ = zxs[d]
                # fresh PSUM slots per (s, d); zifo and zg in separate banks
                zifo = zp.tile([128, LO], FP, tag=f"az{d}",
                               name=f"zi{d}")[:, 0:3]
                zg = zp.tile([128, LO], FP, tag=f"az{2 + d}",
                             name=f"zgt{d}")[:, 0:1]
                # inject first: it has no dependency on h, runs ahead
                nc.tensor.matmul(zifo, lhsT=ident,
                                 rhs=zx_ifo[:, se, :],
                                 start=True, stop=(s == 0),
                                 skip_group_check=True)
                if s > 0:
                    nc.tensor.matmul(zg, lhsT=bdr(d, 3), rhs=hT[d][:],
                                     start=True, stop=True,
                                     skip_group_check=True)
                    for g in range(3):
                        nc.tensor.matmul(
                            zifo[:, g:g + 1], lhsT=bdr(d, g),
                            rhs=hT[d][:], start=False, stop=(g == 2),
                            skip_group_check=True)
                return zifo, zg, se

            for s in range(LO):
                zz = [pb_mm(s, 0), pb_mm(s, 1)]
                # gate cols: 0=i 1=f 2=o 3=g' (sigmoid of 2x)
                tl = []
                for d in range(2):
                    tl.append((gpb.tile([128, 3], BF, tag=f"sg{d}",
                                        name=f"sg{d}"),
                               gpb.tile([128, 1], BF, tag=f"tg{d}",
                                        name=f"tg{d}"),
                               gpb.tile([128, 1], BF, tag=f"tc{d}",
                                        name=f"tc{d}"),
                               gpb.tile([128, 1], FP, tag=f"tm1{d}",
                                        name=f"tm1{d}")))
                # interleave the two chains op-by-op on each engine
                for d in range(2):
                    zifo, zg, se = zz[d]
                    sg, tgl, tcl, tm1 = tl[d]
                    zx_g = zxs[d][1]
                    if s > 0:
                        nc.scalar.activation(tgl[:], zg, AF.Tanh,
                                             bias=zx_g[:, se:se + 1])
                    else:
                        nc.scalar.activation(tgl[:], zx_g[:, se:se + 1],
                                             AF.Tanh)
                    nc.scalar.activation(sg[:], zifo, AF.Sigmoid)
                    # tm1 = sig_i * tanh_g
                    nc.vector.scalar_tensor_tensor(
                        tm1[:], sg[:, 0:1], tgl[:], sg[:, 0:1],
                        ALU.mult, ALU.bypass)
                    if s > 0:
                        nc.vector.scalar_tensor_tensor(
                            cT[d][:], sg[:, 1:2], cT[d][:], tm1[:],
                            ALU.mult, ALU.add)
                    else:
                        nc.vector.tensor_copy(cT[d][:], tm1[:])
                for d in range(2):
                    sg, tgl, tcl, tm1 = tl[d]
                    nc.scalar.activation(tcl[:], cT[d][:], AF.Tanh)
                    nc.vector.scalar_tensor_tensor(
                        hT[d][:], sg[:, 2:3], tcl[:], sg[:, 2:3],
                        ALU.mult, ALU.bypass)

            # ---------------- dense + sigmoid ----------------
            fo = zp.tile([128, LO], FP, tag="az2",
                         name="fo")[0:BL, 0:1]
            nc.tensor.matmul(fo, lhsT=wdx[0], rhs=hT[0][:],
                             start=True, stop=False, skip_group_check=True)
            nc.tensor.matmul(fo, lhsT=wdx[1], rhs=hT[1][:],
                             start=False, stop=True, skip_group_check=True)
            res = gp.tile([BL, 1], FP, tag="res")
            nc.scalar.activation(res[:], fo, AF.Sigmoid, bias=bd)
            nc.sync.dma_start(out=out[:], in_=res[:])

    nc.compile()
    return nc


def _prep_inputs(x, k_conv, r_conv, b_conv, k_f, r_f, b_f, k_b, r_b, b_b,
                 w_d, b_d):
    """Host-side: gate reorder, block-diag expansion, x transpose."""
    assert np.all(b_conv == 0.0), "nonzero b_conv not supported by this kernel"
    k_conv = _reorder_gates(np.asarray(k_conv, np.float32), F)
    r_conv = _reorder_gates(np.asarray(r_conv, np.float32), F)
    k_f = _reorder_gates(np.asarray(k_f, np.float32), U)
    r_f = _reorder_gates(np.asarray(r_f, np.float32), U)
    b_f = _reorder_gates(np.asarray(b_f, np.float32), U)
    k_b = _reorder_gates(np.asarray(k_b, np.float32), U)
    r_b = _reorder_gates(np.asarray(r_b, np.float32), U)
    b_b = _reorder_gates(np.asarray(b_b, np.float32), U)

    import ml_dtypes
    w_bf = np.zeros((128, WBF_COLS), np.float32)
    w_all = np.zeros((128, W_COLS), np.float32)
    for g in range(4):
        for tap in range(2):
            wi = np.zeros((128, 128), np.float32)
            wr = np.zeros((128, 128), np.float32)
            for b in range(4):
                sl = slice(b * 32, (b + 1) * 32)
                wi[sl, sl] = k_conv[tap, :, g * 32:(g + 1) * 32]
                wr[sl, sl] = r_conv[tap, :, g * 32:(g + 1) * 32]
            w_bf[:, (g * 2 + tap) * 128:(g * 2 + tap + 1) * 128] = wi
            w_bf[:, (8 + g * 2 + tap) * 128:(9 + g * 2 + tap) * 128] = wr
    w_bf[:, 2048:2176] = np.eye(128, dtype=np.float32)
    w_d = np.asarray(w_d, np.float32)
    for d, (kk, rr, bb) in enumerate([(k_f, r_f, b_f), (k_b, r_b, b_b)]):
        for g in range(4):
            bk = np.zeros((128, 128), np.float32)
            br = np.zeros((128, 128), np.float32)
            for b in range(4):
                sl = slice(b * 32, (b + 1) * 32)
                bk[sl, sl] = kk[:, g * 32:(g + 1) * 32]
                br[sl, sl] = rr[:, g * 32:(g + 1) * 32]
            w_bf[:, 2176 + (d * 4 + g) * 128:2304 + (d * 4 + g) * 128] = bk
            w_bf[:, 3200 + (d * 4 + g) * 128:3328 + (d * 4 + g) * 128] = br
            w_all[:, d * 4 + g] = np.tile(bb[g * 32:(g + 1) * 32], 4)
        wx = np.zeros((128, 4), np.float32)
        for b in range(4):
            wx[b * 32:(b + 1) * 32, b] = w_d[d * 32:(d + 1) * 32, 0]
        w_bf[:, 4224 + d * 4:4228 + d * 4] = wx
    w_all[:, 8] = 0.5
    w_all[0:4, 9] = np.float32(np.asarray(b_d).reshape(-1)[0])
    w_bf = w_bf.astype(ml_dtypes.bfloat16)

    # x (B,T,512,C) -> per-core (128=(b,c), T, (tap,j)): x2[b*32+c, t, tap*256+j]
    #   = x[b, t, 2j+tap, c]
    x = np.asarray(x, np.float32).reshape(B, T, LO, 2, C)
    # -> (B, C, T, tap, j)
    xt = np.ascontiguousarray(x.transpose(0, 4, 1, 3, 2))
    x2_full = xt.reshape(B * C, T, 2 * LO)

    x2_full = x2_full.astype(ml_dtypes.bfloat16)
    in_maps = []
    for core in range(NCORES):
        x2c = np.ascontiguousarray(
            x2_full[core * BL * C:(core + 1) * BL * C])
        in_maps.append({"x2": x2c, "w_bf": w_bf, "w_all": w_all})
    return in_maps


def kernel(**inputs) -> np.ndarray:
    if "nc" not in _CACHE:
        _CACHE["nc"] = _build_graph()
    nc = _CACHE["nc"]
    in_maps = _prep_inputs(**inputs)
    res = run_bass_kernel_spmd(nc, in_maps, core_ids=list(range(NCORES)))
    outs = [res.results[i]["out"].reshape(BL, 1) for i in range(NCORES)]
    return np.concatenate(outs, axis=0).astype(np.float32)


# revision 28
# speedup vs baseline: 1.6144x; 1.1929x over previous
"""Trainium2 Bass kernel: ConvLSTM1D -> BiLSTM -> dense sigmoid.

Reference model (per full batch B=32):
  h = ConvLSTM1D(x (B,64,512,32); k (2,32,128) stride2, r (2,32,128), hard_sigmoid)
      -> final hidden (B, 256, 32)
  hf = LSTM(h) last state; hb = LSTM(h reversed) last state  (U=32 each)
  out = sigmoid(concat(hf,hb) @ w_d + b_d)   (B, 1)

Sharding: pure data parallelism, batch 32 -> 8 cores x 4.

Per-core layout choices:
  ConvLSTM scan state/gates: partitions = (b4, ch32) = 128, free = j (256).
    Matmuls use block-diagonal weights lhsT[(b',cin),(b,ch)] = delta_bb' W[cin,ch]
    (K=128, M=128, N=256, float32r -> 1 cycle/row) accumulating input-conv taps
    and recurrent-conv taps into one PSUM group per gate.
  BiLSTM: transposed layout, partitions = (gate,U) = 128, free = batch (4).
    Two interleaved chains (fwd, bwd); zx injected by identity-matmul.
Gate order is host-reordered from Keras (i,f,g,o) to (i,f,o,g) so the three
hard-sigmoid/sigmoid gates are contiguous.
"""

import numpy as np

import concourse.bass as bass
import concourse.bacc as bacc
import concourse.mybir as mybir
from concourse.tile import TileContext
from concourse.bass_utils import run_bass_kernel_spmd

B, T, L, C = 32, 64, 512, 32
F = 32          # conv filters
U = 32          # lstm units
NCORES = 8
BL = B // NCORES          # 4 local batch
LO = L // 2               # 256 spatial after stride-2 conv
G4 = 4 * F                # 128 gate channels

FP = mybir.dt.float32
BF = mybir.dt.bfloat16

# w_bf column layout (bf16): big matmul weights
#  [0:2048)    16 block-diag (128x128) conv weights, index (g*2+tap)*128,
#              first 8 = input conv, next 8 = recurrent conv
#  [2048:2176) identity 128x128
#  [2176:3200) 8 block-diag zx weights bdk[d][g][(b,ch),(b,U)]
#  [3200:4224) 8 block-diag lstm rec weights bdr[d][g][(b,U'),(b,U)]
#  [4224:4232) dense wdx[d] (128,4): [(b,u), b] = delta * w_d[u+32d]
WBF_COLS = 4232
# w_all column layout (f32): biases
#  [0:8)       lstm biases per (d,g): (128,1) = b_d[g*32+u]
#  [8]         0.5 constant
#  [9]         b_d (dense bias) replicated
W_COLS = 10

_CACHE = {}


def _reorder_gates(w, n):
    # last dim (4n): keras order i,f,g,o -> i,f,o,g
    i, f, g, o = np.split(w, 4, axis=-1)
    return np.concatenate([i, f, o, g], axis=-1)


def _build_graph():
    nc = bacc.Bacc("TRN2")
    x2 = nc.declare_dram_parameter("x2", [128, T, 2 * LO], BF, isOutput=False)
    w_bf = nc.declare_dram_parameter("w_bf", [128, WBF_COLS], BF, isOutput=False)
    w_all = nc.declare_dram_parameter("w_all", [128, W_COLS], FP, isOutput=False)
    out = nc.declare_dram_parameter("out", [BL, 1], FP, isOutput=True)

    AF = mybir.ActivationFunctionType
    ALU = mybir.AluOpType

    with TileContext(nc) as tc:
        with (
            tc.tile_pool(name="w", bufs=1) as wp,
            tc.tile_pool(name="x", bufs=4) as xp,
            tc.tile_pool(name="st", bufs=1) as sp,
            tc.tile_pool(name="g", bufs=3) as gp,
            tc.tile_pool(name="gb", bufs=8) as gpb,
            tc.tile_pool(name="zp", bufs=2, space="PSUM") as zp,
        ):
            W = wp.tile([128, W_COLS], FP)
            nc.sync.dma_start(out=W[:], in_=w_all[:])
            WB = wp.tile([128, WBF_COLS], BF)
            nc.sync.dma_start(out=WB[:], in_=w_bf[:])

            def wconv(idx):  # (128,128) bf16 block-diag conv weight
                return WB[:, idx * 128:(idx + 1) * 128]

            ident = WB[:, 2048:2176]

            def bdk(d, g):  # zx input weights, block-diag (bf16)
                o = 2176 + (d * 4 + g) * 128
                return WB[:, o:o + 128]

            def bdr(d, g):  # lstm recurrent weights, block-diag (bf16)
                o = 3200 + (d * 4 + g) * 128
                return WB[:, o:o + 128]

            wdx = [WB[:, 4224:4228], WB[:, 4228:4232]]
            bls = [[W[:, d * 4 + g:d * 4 + g + 1] for g in range(4)]
                   for d in range(2)]
            half = W[:, 8:9]
            bd = W[0:4, 9:10]

            # ---------------- Phase A: ConvLSTM scan over T ----------------
            h_sb = sp.tile([128, LO + 1], BF)   # col 256 stays zero (pad)
            c_sb = sp.tile([128, LO], FP)
            nc.vector.memset(h_sb[:, LO:LO + 1], 0.0)

            # two PSUM tiles (one bank each) so gate reads never falsely
            # serialize against later gates' matmul writes:
            # zA = [g | f], zB = [i | o]; emission order g, i, f, o
            for t in range(T):
                xt = xp.tile([128, 2, LO], BF, tag="xt")
                nc.sync.dma_start(out=xt[:], in_=x2[:, t, :])
                zt4 = [zp.tile([128, LO], FP, tag=f"az{g}",
                               name=f"az{g}") for g in range(4)]
                sig = gp.tile([128, 3, LO], BF, tag="sig")
                tg = gp.tile([128, LO], BF, tag="tg")
                tc_t = gp.tile([128, LO], BF, tag="tc")
                tmp = gp.tile([128, LO], BF, tag="tmp")
                c2 = gp.tile([128, LO], FP, tag="c2")

                def conv_inp(g, zg):
                    for tap in range(2):
                        nc.tensor.matmul(
                            zg[:], lhsT=wconv(g * 2 + tap), rhs=xt[:, tap, :],
                            start=(tap == 0),
                            stop=(t == 0 and tap == 1))

                def conv_rec(g, zg):
                    for tap in range(2):
                        nc.tensor.matmul(
                            zg[:], lhsT=wconv(8 + g * 2 + tap),
                            rhs=h_sb[:, tap:tap + LO],
                            start=False, stop=(tap == 1))

                # gate index in weights: 0=i 1=f 2=o 3=g (host order i,f,o,g)
                # psum tile index: zt4[0]=g zt4[1]=i zt4[2]=f zt4[3]=o
                # all input-side matmuls first: they have no h dependency, so
                # the in-order PE queue fills the previous step's gate tail
                for g_, p_ in ((3, 0), (0, 1), (1, 2), (2, 3)):
                    conv_inp(g_, zt4[p_])
                if t > 0:
                    conv_rec(3, zt4[0])
                nc.scalar.activation(tg[:], zt4[0][:], AF.Tanh)
                if t > 0:
                    conv_rec(0, zt4[1])
                nc.scalar.activation(sig[:, 0, :], zt4[1][:],
                                     AF.Relu, bias=half, scale=0.2)
                # tmp = min(sig_i,1) * tanh(zg)
                nc.vector.scalar_tensor_tensor(
                    (c_sb[:] if t == 0 else tmp[:]),
                    sig[:, 0, :], 1.0, tg[:], ALU.min, ALU.mult)
                if t > 0:
                    conv_rec(1, zt4[2])
                nc.scalar.activation(sig[:, 1, :], zt4[2][:],
                                     AF.Relu, bias=half, scale=0.2)
                if t > 0:
                    nc.vector.scalar_tensor_tensor(
                        c2[:], sig[:, 1, :], 1.0, c_sb[:], ALU.min, ALU.mult)
                    nc.vector.tensor_tensor(c_sb[:], tmp[:], c2[:], ALU.add)
                if t > 0:
                    conv_rec(2, zt4[3])
                nc.scalar.activation(sig[:, 2, :], zt4[3][:],
                                     AF.Relu, bias=half, scale=0.2)
                nc.scalar.activation(tc_t[:], c_sb[:], AF.Tanh)
                nc.vector.scalar_tensor_tensor(
                    h_sb[:, 0:LO], sig[:, 2, :], 1.0, tc_t[:],
                    ALU.min, ALU.mult)

            # ---------------- Phase B: bidirectional LSTM over LO ----------
            # Layout: partitions = (b,U) = 128, free = gate cols. No partition
            # shifts anywhere (walrus verifier requires same partitions).
            # zx[d][g] (128, LO): input-side gates + lstm bias, injected into
            # the per-step PSUM via identity matmul (i,f,o) / ACT bias (g).
            zxs = []
            for d in range(2):
                pss = [zp.tile([128, LO], FP, tag=f"az{g}",
                               name=f"zxps{g}") for g in range(4)]

                def ps_slice(g):
                    return pss[g][:]

                for g in range(4):
                    nc.tensor.matmul(
                        ps_slice(g), lhsT=bdk(d, g),
                        rhs=h_sb[:, 0:LO],
                        start=True, stop=True)
                zx_ifo = sp.tile([128, LO, 3], BF, tag=f"zxifo{d}",
                                 name=f"zxifo{d}")
                zx_g = sp.tile([128, LO], FP, tag=f"zxg{d}", name=f"zxg{d}")
                # evacuation + lstm-bias fold; split across ACT and DVE
                nc.scalar.activation(
                    zx_ifo[:, :, 0], ps_slice(0), AF.Identity, bias=bls[d][0])
                nc.vector.scalar_tensor_tensor(
                    zx_ifo[:, :, 1], ps_slice(1), bls[d][1],
                    h_sb[:, 0:LO], ALU.add, ALU.bypass)
                nc.scalar.activation(
                    zx_ifo[:, :, 2], ps_slice(2), AF.Identity,
                    bias=bls[d][2])
                nc.vector.scalar_tensor_tensor(
                    zx_g[:], ps_slice(3), bls[d][3],
                    h_sb[:, 0:LO], ALU.add, ALU.bypass)
                zxs.append((zx_ifo, zx_g))

            # state: hT[d] bf16 (feeds bf16 matmul), cT[d] f32
            hT = [sp.tile([128, 1], BF, tag=f"hT{d}", name=f"hT{d}")
                  for d in range(2)]
            cT = [sp.tile([128, 1], FP, tag=f"cT{d}", name=f"cT{d}")
                  for d in range(2)]

            def pb_mm(s, d):
                se = s if d == 0 else LO - 1 - s
                zx_ifo, _ = zxs[d]
                # fresh PSUM slots per (s, d); zifo and zg in separate banks
                zifo = zp.tile([128, LO], FP, tag=f"az{d}",
                               name=f"zi{d}")[:, 0:3]
                zg = zp.tile([128, LO], FP, tag=f"az{2 + d}",
                             name=f"zgt{d}")[:, 0:1]
                # inject first: it has no dependency on h, runs ahead
                nc.tensor.matmul(zifo, lhsT=ident,
                                 rhs=zx_ifo[:, se, :],
                                 start=True, stop=(s == 0),
                                 skip_group_check=True)
                if s > 0:
                    nc.tensor.matmul(zg, lhsT=bdr(d, 3), rhs=hT[d][:],
                                     start=True, stop=True,
                                     skip_group_check=True)
                    for g in range(3):
                        nc.tensor.matmul(
                            zifo[:, g:g + 1], lhsT=bdr(d, g),
                            rhs=hT[d][:], start=False, stop=(g == 2),
                            skip_group_check=True)
                return zifo, zg, se

            for s in range(LO):
                zz = [pb_mm(s, 0), pb_mm(s, 1)]
                # gate cols: 0=i 1=f 2=o 3=g' (sigmoid of 2x)
                tl = []
                for d in range(2):
                    tl.append((gpb.tile([128, 2], BF, tag=f"sg{d}",
                                        name=f"sg{d}"),
                               gpb.tile([128, 1], BF, tag=f"so{d}",
                                        name=f"so{d}"),
                               gpb.tile([128, 1], BF, tag=f"tg{d}",
                                        name=f"tg{d}"),
                               gpb.tile([128, 1], BF, tag=f"tc{d}",
                                        name=f"tc{d}"),
                               gpb.tile([128, 1], FP, tag=f"tm1{d}",
                                        name=f"tm1{d}")))
                # interleave the two chains op-by-op on each engine
                for d in range(2):
                    zifo, zg, se = zz[d]
                    sg, so, tgl, tcl, tm1 = tl[d]
                    zx_g = zxs[d][1]
                    if s > 0:
                        nc.scalar.activation(tgl[:], zg, AF.Tanh,
                                             bias=zx_g[:, se:se + 1])
                    else:
                        nc.scalar.activation(tgl[:], zx_g[:, se:se + 1],
                                             AF.Tanh)
                    # deep-chain gates (i, f) first; o off the critical path
                    nc.scalar.activation(sg[:], zifo[:, 0:2], AF.Sigmoid)
                    # tm1 = sig_i * tanh_g
                    nc.vector.scalar_tensor_tensor(
                        tm1[:], sg[:, 0:1], tgl[:], sg[:, 0:1],
                        ALU.mult, ALU.bypass)
                    if s > 0:
                        nc.vector.scalar_tensor_tensor(
                            cT[d][:], sg[:, 1:2], cT[d][:], tm1[:],
                            ALU.mult, ALU.add)
                    else:
                        nc.vector.tensor_copy(cT[d][:], tm1[:])
                    nc.scalar.activation(so[:], zifo[:, 2:3], AF.Sigmoid)
                for d in range(2):
                    sg, so, tgl, tcl, tm1 = tl[d]
                    nc.scalar.activation(tcl[:], cT[d][:], AF.Tanh)
                    nc.vector.scalar_tensor_tensor(
                        hT[d][:], so[:, 0:1], tcl[:], so[:, 0:1],
                        ALU.mult, ALU.bypass)

            # ---------------- dense + sigmoid ----------------
            fo = zp.tile([128, LO], FP, tag="az2",
                         name="fo")[0:BL, 0:1]
            nc.tensor.matmul(fo, lhsT=wdx[0], rhs=hT[0][:],
                             start=True, stop=False, skip_group_check=True)
            nc.tensor.matmul(fo, lhsT=wdx[1], rhs=hT[1][:],
                             start=False, stop=True, skip_group_check=True)
            res = gp.tile([BL, 1], FP, tag="res")
            nc.scalar.activation(res[:], fo, AF.Sigmoid, bias=bd)
            nc.sync.dma_start(out=out[:], in_=res[:])

    nc.compile()
    return nc


def _prep_inputs(x, k_conv, r_conv, b_conv, k_f, r_f, b_f, k_b, r_b, b_b,
                 w_d, b_d):
    """Host-side: gate reorder, block-diag expansion, x transpose."""
    assert np.all(b_conv == 0.0), "nonzero b_conv not supported by this kernel"
    k_conv = _reorder_gates(np.asarray(k_conv, np.float32), F)
    r_conv = _reorder_gates(np.asarray(r_conv, np.float32), F)
    k_f = _reorder_gates(np.asarray(k_f, np.float32), U)
    r_f = _reorder_gates(np.asarray(r_f, np.float32), U)
    b_f = _reorder_gates(np.asarray(b_f, np.float32), U)
    k_b = _reorder_gates(np.asarray(k_b, np.float32), U)
    r_b = _reorder_gates(np.asarray(r_b, np.float32), U)
    b_b = _reorder_gates(np.asarray(b_b, np.float32), U)

    import ml_dtypes
    w_bf = np.zeros((128, WBF_COLS), np.float32)
    w_all = np.zeros((128, W_COLS), np.float32)
    for g in range(4):
        for tap in range(2):
            wi = np.zeros((128, 128), np.float32)
            wr = np.zeros((128, 128), np.float32)
            for b in range(4):
                sl = slice(b * 32, (b + 1) * 32)
                wi[sl, sl] = k_conv[tap, :, g * 32:(g + 1) * 32]
                wr[sl, sl] = r_conv[tap, :, g * 32:(g + 1) * 32]
            w_bf[:, (g * 2 + tap) * 128:(g * 2 + tap + 1) * 128] = wi
            w_bf[:, (8 + g * 2 + tap) * 128:(9 + g * 2 + tap) * 128] = wr
    w_bf[:, 2048:2176] = np.eye(128, dtype=np.float32)
    w_d = np.asarray(w_d, np.float32)
    for d, (kk, rr, bb) in enumerate([(k_f, r_f, b_f), (k_b, r_b, b_b)]):
        for g in range(4):
            bk = np.zeros((128, 128), np.float32)
            br = np.zeros((128, 128), np.float32)
            for b in range(4):
                sl = slice(b * 32, (b + 1) * 32)
                bk[sl, sl] = kk[:, g * 32:(g + 1) * 32]
                br[sl, sl] = rr[:, g * 32:(g + 1) * 32]
            w_bf[:, 2176 + (d * 4 + g) * 128:2304 + (d * 4 + g) * 128] = bk
            w_bf[:, 3200 + (d * 4 + g) * 128:3328 + (d * 4 + g) * 128] = br
            w_all[:, d * 4 + g] = np.tile(bb[g * 32:(g + 1) * 32], 4)
        wx = np.zeros((128, 4), np.float32)
        for b in range(4):
            wx[b * 32:(b + 1) * 32, b] = w_d[d * 32:(d + 1) * 32, 0]
        w_bf[:, 4224 + d * 4:4228 + d * 4] = wx
    w_all[:, 8] = 0.5
    w_all[0:4, 9] = np.float32(np.asarray(b_d).reshape(-1)[0])
    w_bf = w_bf.astype(ml_dtypes.bfloat16)

    # x (B,T,512,C) -> per-core (128=(b,c), T, (tap,j)): x2[b*32+c, t, tap*256+j]
    #   = x[b, t, 2j+tap, c]
    x = np.asarray(x, np.float32).reshape(B, T, LO, 2, C)
    # -> (B, C, T, tap, j)
    xt = np.ascontiguousarray(x.transpose(0, 4, 1, 3, 2))
    x2_full = xt.reshape(B * C, T, 2 * LO)

    x2_full = x2_full.astype(ml_dtypes.bfloat16)
    in_maps = []
    for core in range(NCORES):
        x2c = np.ascontiguousarray(
            x2_full[core * BL * C:(core + 1) * BL * C])
        in_maps.append({"x2": x2c, "w_bf": w_bf, "w_all": w_all})
    return in_maps


def kernel(**inputs) -> np.ndarray:
    if "nc" not in _CACHE:
        _CACHE["nc"] = _build_graph()
    nc = _CACHE["nc"]
    in_maps = _prep_inputs(**inputs)
    res = run_bass_kernel_spmd(nc, in_maps, core_ids=list(range(NCORES)))
    outs = [res.results[i]["out"].reshape(BL, 1) for i in range(NCORES)]
    return np.concatenate(outs, axis=0).astype(np.float32)
